# revision 10
# baseline (speedup 1.0000x reference)
"""Trainium2 Bass kernel: varlen batched cross-attention (sparse_attention).

Math (per reference):
  qh = q @ Wq.T           [Tq, H, D]
  k,v = split(x @ Wkv.T)  [B, N, H, D]
  per batch b: queries of segment b attend over batch b's N keys
  out = softmax(qh k^T / sqrt(D)) v  -> [Tq, C] @ Wproj.T + bproj

Sharding: batch-parallel over 8 cores (core b owns batch b), zero
collectives. Host pre-transposes operands; device matmuls contract
over the partition axis. Queries padded to uniform L so one NEFF
serves all cores.

V2 engine plan (per core):
  scores  S^T = K_h^T Q_h per 128-key tile, head-paired row tiles
  exp     split: most key-tiles on ScalarE (f=1536 activates), first
          NT_DVE tiles on VectorE via Schraudolph int16/bf16 bitcast
  Z       VectorE tensor_tensor accumulation over key tiles, then
          ones-matmul broadcast + fast reciprocal
  attnV   col-tiled matmul pairs (even nt -> psum[0:64] @(0,0),
          odd nt -> psum[64:128] @(0,64)) -- concurrent streams, 2x
  yproj   per 384-col chunk, heads chained in PSUM, bias fused
"""

import os
import numpy as np

B, NKEY, C, H, D = 8, 2048, 512, 8, 64
NCORES = 8
CT = C // 128          # 4 c-tiles
NT = NKEY // 128       # 16 key tiles
HPAIRS = H // 2        # 4 head pairs
SCALE = float(D) ** -0.5
NT_DVE = 5             # key tiles exp'd on VectorE (Schraudolph)

# Schraudolph-to-bf16 constants: bf16bits = round(A*x + B), x = raw score
LOG2E = 1.4426950408889634
SCH_A = 128.0 * LOG2E * SCALE
SCH_B = 127.0 * 128.0 - 7.0

_BUILD_CACHE = {}


def _halves(L):
    # exp-granularity l-chunks (~768) and attnV sub-chunks (384)
    out = []
    off = 0
    while off < L:
        sz = min(768, L - off)
        out.append((off, sz))
        off += sz
    return out


def _build(L, dbg=False):
    key = (L, dbg)
    if key in _BUILD_CACHE:
        return _BUILD_CACHE[key]
    from contextlib import ExitStack
    import concourse.bass as bass
    import concourse.tile as tile
    import concourse.mybir as mybir
    from concourse import bacc

    f32 = mybir.dt.float32
    bf16 = mybir.dt.bfloat16
    i16 = mybir.dt.int16
    AF = mybir.ActivationFunctionType
    ALU = mybir.AluOpType

    lch = _halves(L)            # [(0,768),(768,768),(1536,384)] for L=1920

    nc = bacc.Bacc("TRN2", target_bir_lowering=False, debug=False)
    xT = nc.declare_dram_parameter("xT", [C, NKEY], bf16, isOutput=False)
    qT = nc.declare_dram_parameter("qT", [C, L], bf16, isOutput=False)
    wqT = nc.declare_dram_parameter("wqT", [C, C], bf16, isOutput=False)
    wkT = nc.declare_dram_parameter("wkT", [C, C], bf16, isOutput=False)
    wvT = nc.declare_dram_parameter("wvT", [C, C], bf16, isOutput=False)
    wpT = nc.declare_dram_parameter("wpT", [C, C], bf16, isOutput=False)
    biasP = nc.declare_dram_parameter("biasP", [128, CT], f32, isOutput=False)
    outT = nc.declare_dram_parameter("out", [C, L], f32, isOutput=True)

    with ExitStack() as ctx:
        tc = ctx.enter_context(tile.TileContext(nc))
        pers = ctx.enter_context(tc.tile_pool(name="pers", bufs=1))
        # scores psum: 2 x [128,1536] f32 = 6 banks
        psS = ctx.enter_context(tc.tile_pool(name="psS", bufs=1, space="PSUM"))
        # small psum: attnV poz / z-broadcast / yproj, [128,<=512] = 2 banks
        psZ = ctx.enter_context(tc.tile_pool(name="psZ", bufs=4, space="PSUM"))
        ptp = ctx.enter_context(tc.tile_pool(name="ptp", bufs=18))
        work = ctx.enter_context(tc.tile_pool(name="work", bufs=3))

        # ---- persistent inputs -------------------------------------------
        xt_sb = [pers.tile([128, NKEY], bf16, tag=f"xt{i}", name=f"xt{i}") for i in range(CT)]
        qt_sb = [pers.tile([128, L], bf16, tag=f"qt{i}", name=f"qt{i}") for i in range(CT)]
        wq_sb = [pers.tile([128, C], bf16, tag=f"wq{i}", name=f"wq{i}") for i in range(CT)]
        wk_sb = [pers.tile([128, C], bf16, tag=f"wk{i}", name=f"wk{i}") for i in range(CT)]
        wv_sb = [pers.tile([128, C], bf16, tag=f"wv{i}", name=f"wv{i}") for i in range(CT)]
        wp_sb = [pers.tile([128, C], bf16, tag=f"wp{i}", name=f"wp{i}") for i in range(CT)]
        for i in range(CT):
            sl = slice(128 * i, 128 * (i + 1))
            nc.sync.dma_start(xt_sb[i][:], xT[sl, :])
            nc.sync.dma_start(wk_sb[i][:], wkT[sl, :])
            nc.sync.dma_start(wv_sb[i][:], wvT[sl, :])
        for i in range(CT):
            sl = slice(128 * i, 128 * (i + 1))
            nc.sync.dma_start(wq_sb[i][:], wqT[sl, :])
            nc.sync.dma_start(qt_sb[i][:], qT[sl, :])
            nc.sync.dma_start(wp_sb[i][:], wpT[sl, :])
        bias_sb = pers.tile([128, CT], f32, tag="bias")
        nc.sync.dma_start(bias_sb[:], biasP[:])

        ones_sb = pers.tile([128, 128], bf16, tag="ones")
        nc.vector.memset(ones_sb[:], 1.0)

        # ---- projections --------------------------------------------------
        kt_sb = [pers.tile([128, NKEY], bf16, tag=f"kt{i}", name=f"kt{i}") for i in range(HPAIRS)]
        qht_sb = [pers.tile([128, L], bf16, tag=f"qht{i}", name=f"qht{i}") for i in range(HPAIRS)]
        v_sb = [pers.tile([128, H * D], bf16, tag=f"va{i}", name=f"va{i}") for i in range(NT)]

        def proj_kt(jt):
            for nch in range(NKEY // 512):
                ps = psZ.tile([128, 512], f32, tag="psZ")
                for ct in range(CT):
                    nc.tensor.matmul(
                        ps[:, 0:512],
                        lhsT=wk_sb[ct][:, 128 * jt:128 * (jt + 1)],
                        rhs=xt_sb[ct][:, 512 * nch:512 * (nch + 1)],
                        start=(ct == 0), stop=(ct == CT - 1))
                nc.vector.tensor_copy(
                    kt_sb[jt][:, 512 * nch:512 * (nch + 1)], ps[:, 0:512])

        def proj_qht(jt):
            for qch in range((L + 511) // 512):
                qs, qn = 512 * qch, min(512, L - 512 * qch)
                ps = psZ.tile([128, 512], f32, tag="psZ")
                for ct in range(CT):
                    nc.tensor.matmul(
                        ps[:, 0:qn],
                        lhsT=wq_sb[ct][:, 128 * jt:128 * (jt + 1)],
                        rhs=qt_sb[ct][:, qs:qs + qn],
                        start=(ct == 0), stop=(ct == CT - 1))
                nc.vector.tensor_copy(qht_sb[jt][:, qs:qs + qn], ps[:, 0:qn])

        def proj_v(nt):
            ps = psZ.tile([128, 512], f32, tag="psZ")
            for ct in range(CT):
                nc.tensor.matmul(
                    ps[:, 0:512],
                    lhsT=xt_sb[ct][:, 128 * nt:128 * (nt + 1)],
                    rhs=wv_sb[ct][:, 0:C],
                    start=(ct == 0), stop=(ct == CT - 1))
            nc.vector.tensor_copy(v_sb[nt][:], ps[:, 0:512])

        # ---- attention: one (head-pair, l-half) group --------------------
        def attn_scores(hp, lc_i):
            """scores + exp for all 16 key tiles of (hp, half lc_i).
            Returns list of pt tiles [128, 2*768] (h1 cols 0:768, h2 768:)
            and the zacc tile."""
            lcs, lcn = lch[lc_i]
            pts = []
            zacc = None
            for nt in range(NT):
                nsl = slice(128 * nt, 128 * (nt + 1))
                ps = psS.tile([128, 1536], f32, tag="psS")
                # matmul PSUM writes must stay within one 512-f32 bank:
                # split each head's [hh*768, hh*768+lcn) range at absolute
                # 512-col boundaries of the psum tile
                for hh in range(2):
                    a = hh * 768
                    end = a + lcn
                    while a < end:
                        b = min(end, (a // 512 + 1) * 512)
                        qa = lcs + (a - hh * 768)
                        nc.tensor.matmul(
                            ps[:, a:b],
                            lhsT=kt_sb[hp][64 * hh:64 * hh + 64, nsl],
                            rhs=qht_sb[hp][64 * hh:64 * hh + 64, qa:qa + (b - a)],
                            start=True, stop=True,
                            tile_position=(64 * hh, 0))
                        a = b
                pt = ptp.tile([128, 1536], bf16, tag="pt")
                ps3 = ps[:].rearrange("p (b x) -> p b x", b=2)
                pt3 = pt[:].rearrange("p (b x) -> p b x", b=2)
                if nt < NT_DVE:
                    pti = pt[:].bitcast(i16).rearrange("p (b x) -> p b x", b=2)
                    nc.vector.tensor_scalar(
                        pti[:, :, 0:lcn], ps3[:, :, 0:lcn],
                        SCH_A, SCH_B, ALU.mult, ALU.add)
                else:
                    nc.scalar.activation(pt3[:, :, 0:lcn], ps3[:, :, 0:lcn],
                                         AF.Exp, scale=SCALE)
                pts.append(pt)
                if nt == 1:
                    zacc = work.tile([128, 1536], bf16, tag="zacc")
                    nc.vector.tensor_tensor(zacc[:], pts[0][:], pts[1][:],
                                            ALU.add)
                elif nt > 1:
                    nc.vector.tensor_tensor(zacc[:], zacc[:], pts[nt][:],
                                            ALU.add)
            return pts, zacc

        def attn_v(hp, lc_i, pts, zacc):
            """attnV + normalize for both heads of hp over half lc_i.
            Returns onrm tile [128, lcn] (h1 rows 0:64, h2 64:128)."""
            lcs, lcn = lch[lc_i]
            onrm = work.tile([128, 768], bf16, tag=f"onrm{hp}", name=f"onrm{hp}")
            for sub in range(0, lcn, 384):
                sn = min(384, lcn - sub)
                for hh in range(2):
                    h = 2 * hp + hh
                    hoff = hh * 768
                    poz = psZ.tile([128, 384], f32, tag="psZ")
                    BISECT_PAIRED = True
                    for nt in range(NT):
                        half = (nt % 2) if BISECT_PAIRED else 0
                        nc.tensor.matmul(
                            poz[64 * half:64 * half + 64, 0:sn],
                            lhsT=v_sb[nt][:, 64 * h:64 * h + 64],
                            rhs=pts[nt][:, hoff + sub:hoff + sub + sn],
                            start=(nt < 2) if BISECT_PAIRED else (nt == 0),
                            stop=(nt >= NT - 2) if BISECT_PAIRED else (nt == NT - 1),
                            tile_position=(0, 64 * half),
                            skip_group_check=True)
                    # Z broadcast: ones^T . zacc -> all 128 partitions
                    pbz = psZ.tile([128, 384], f32, tag="psZ")
                    nc.tensor.matmul(
                        pbz[:, 0:sn], lhsT=ones_sb[:, 0:128],
                        rhs=zacc[:, hoff + sub:hoff + sub + sn],
                        start=True, stop=True)
                    bz = work.tile([128, 384], f32, tag="bz")
                    nc.vector.reciprocal_approx_fast(bz[:, 0:sn], pbz[:, 0:sn])
                    # TT cannot read two PSUM operands: stage slot1 via SBUF
                    if False:  # paired: slot add needed
                        nc.vector.tensor_tensor(
                            onrm[64 * hh:64 * hh + 64, sub:sub + sn],
                            poz[0:64, 0:sn], bz[0:64, 0:sn], ALU.mult)
                    else:
                        o1 = work.tile([64, 384], f32, tag="o1")
                        nc.vector.tensor_copy(o1[:, 0:sn], poz[64:128, 0:sn])
                        osum = work.tile([64, 384], f32, tag="osum")
                        nc.vector.tensor_tensor(osum[:, 0:sn], poz[0:64, 0:sn],
                                                o1[:, 0:sn], ALU.add)
                        nc.vector.tensor_tensor(
                            onrm[64 * hh:64 * hh + 64, sub:sub + sn],
                            osum[:, 0:sn], bz[0:64, 0:sn], ALU.mult)
            return onrm

        def proj_out(lc_i, onrms):
            lcs, lcn = lch[lc_i]
            for sub in range(0, lcn, 384):
                sn = min(384, lcn - sub)
                for jt in range(CT):
                    py = psZ.tile([128, 384], f32, tag="psZ")
                    ys = work.tile([128, 384], f32, tag="ys")
                    for hp in range(HPAIRS):
                        nc.tensor.matmul(
                            py[:, 0:sn],
                            lhsT=wp_sb[hp][:, 128 * jt:128 * (jt + 1)],
                            rhs=onrms[hp][:, sub:sub + sn],
                            start=(hp == 0), stop=(hp == HPAIRS - 1))
                    nc.vector.tensor_scalar(
                        ys[:, 0:sn], py[:, 0:sn],
                        bias_sb[:, jt:jt + 1], None, ALU.add)
                    nc.sync.dma_start(
                        outT[128 * jt:128 * (jt + 1),
                             lcs + sub:lcs + sub + sn],
                        ys[:, 0:sn])

        # ---- emission order ----------------------------------------------
        proj_kt(0)
        proj_qht(0)
        proj_kt(1)
        proj_qht(1)
        with tc.high_priority(offset=-(10 ** 6)):
            for nt in range(NT):
                proj_v(nt)
            for jt in range(2, HPAIRS):
                proj_kt(jt)
                proj_qht(jt)
        pending = None
        for lc_i in range(len(lch)):
            onrms = []
            for hp in range(HPAIRS):
                pts, zacc = attn_scores(hp, lc_i)
                onrms.append(attn_v(hp, lc_i, pts, zacc))
                if hp == 0 and pending is not None:
                    proj_out(*pending)
                    pending = None
            pending = (lc_i, onrms)
        proj_out(*pending)

    nc.compile()
    _BUILD_CACHE[key] = nc
    return nc


def kernel(x, q, Wq, Wkv, Wproj, bproj, q_lengths, max_q_len):
    import ml_dtypes
    from concourse.bass_utils import run_bass_kernel_spmd

    bf16 = ml_dtypes.bfloat16
    x = np.asarray(x, np.float32)
    q = np.asarray(q, np.float32)
    Wq = np.asarray(Wq, np.float32)
    Wkv = np.asarray(Wkv, np.float32)
    Wproj = np.asarray(Wproj, np.float32)
    bproj = np.asarray(bproj, np.float32)
    q_lengths = np.asarray(q_lengths, np.int64)
    assert x.shape[0] == NCORES == B

    L = int(((q_lengths.max() + 127) // 128) * 128)
    nc = _build(L)

    offs = np.concatenate([[0], np.cumsum(q_lengths)])
    wqT = np.ascontiguousarray(Wq.T).astype(bf16)
    wkT = np.ascontiguousarray(Wkv[:C].T).astype(bf16)
    wvT = np.ascontiguousarray(Wkv[C:].T).astype(bf16)
    wpT = np.ascontiguousarray(Wproj.T).astype(bf16)
    biasP = np.ascontiguousarray(bproj.reshape(CT, 128).T).astype(np.float32)

    in_maps = []
    for b in range(B):
        Lb = int(q_lengths[b])
        qseg = q[offs[b]:offs[b] + Lb]
        qTp = np.zeros((C, L), bf16)
        qTp[:, :Lb] = qseg.T.astype(bf16)
        in_maps.append({
            "xT": np.ascontiguousarray(x[b].T).astype(bf16),
            "qT": qTp,
            "wqT": wqT, "wkT": wkT, "wvT": wvT, "wpT": wpT,
            "biasP": biasP,
        })

    trace = os.environ.get("KERNEL_TRACE", "") == "1"
    if trace:
        try:
            import sys
            import types
            import antenv
            if "antenv.axon_hooks" not in sys.modules:
                from trn_agent_boot.trn_boot import _ntff_profile_via_ctypes
                hook = _ntff_profile_via_ctypes("/opt/axon/libaxon_pjrt.so")
                mod = types.ModuleType("antenv.axon_hooks")
                mod.get_axon_ntff_profile_hook = lambda: hook
                sys.modules["antenv.axon_hooks"] = mod
                antenv.axon_hooks = mod
        except Exception as e:
            print(f"ntff hook setup failed: {e}")
            trace = False
    res = run_bass_kernel_spmd(nc, in_maps, core_ids=list(range(NCORES)),
                               trace=trace)
    if trace and res.exec_time_ns is not None:
        print(f"HW exec time: {res.exec_time_ns} ns")
        if res.instructions_and_trace:
            print(f"trace: {res.instructions_and_trace[1]}")

    out = np.empty((int(offs[-1]), C), np.float32)
    for b in range(B):
        Lb = int(q_lengths[b])
        out[offs[b]:offs[b] + Lb] = res.results[b]["out"][:, :Lb].T
    return out


# revision 13
# speedup vs baseline: 1.2799x; 1.2799x over previous
"""Trainium2 Bass kernel: varlen batched cross-attention (sparse_attention).

Math (per reference):
  qh = q @ Wq.T           [Tq, H, D]
  k,v = split(x @ Wkv.T)  [B, N, H, D]
  per batch b: queries of segment b attend over batch b's N keys
  out = softmax(qh k^T / sqrt(D)) v  -> [Tq, C] @ Wproj.T + bproj

Sharding: batch-parallel over 8 cores (core b owns batch b), zero
collectives. Host pre-transposes operands; device matmuls contract
over the partition axis. Queries padded to uniform L so one NEFF
serves all cores.

V2 engine plan (per core):
  scores  S^T = K_h^T Q_h per 128-key tile, head-paired row tiles
  exp     split: most key-tiles on ScalarE (f=1536 activates), first
          NT_DVE tiles on VectorE via Schraudolph int16/bf16 bitcast
  Z       VectorE tensor_tensor accumulation over key tiles, then
          ones-matmul broadcast + fast reciprocal
  attnV   col-tiled matmul pairs (even nt -> psum[0:64] @(0,0),
          odd nt -> psum[64:128] @(0,64)) -- concurrent streams, 2x
  yproj   per 384-col chunk, heads chained in PSUM, bias fused
"""

import os
import numpy as np

B, NKEY, C, H, D = 8, 2048, 512, 8, 64
NCORES = 8
CT = C // 128          # 4 c-tiles
NT = NKEY // 128       # 16 key tiles
HPAIRS = H // 2        # 4 head pairs
SCALE = float(D) ** -0.5
NT_DVE = 5             # key tiles exp'd on VectorE (Schraudolph)

# Schraudolph-to-bf16 constants: bf16bits = round(A*x + B), x = raw score
LOG2E = 1.4426950408889634
SCH_A = 128.0 * LOG2E * SCALE
SCH_B = 127.0 * 128.0 - 7.0

_BUILD_CACHE = {}


def _halves(L):
    # l-chunks of 512 (PSUM-bank-aligned for scores/attnV/yproj)
    out = []
    off = 0
    while off < L:
        sz = min(512, L - off)
        out.append((off, sz))
        off += sz
    return out


def _build(L, dbg=False):
    key = (L, dbg)
    if key in _BUILD_CACHE:
        return _BUILD_CACHE[key]
    from contextlib import ExitStack
    import concourse.bass as bass
    import concourse.tile as tile
    import concourse.mybir as mybir
    from concourse import bacc

    f32 = mybir.dt.float32
    bf16 = mybir.dt.bfloat16
    i16 = mybir.dt.int16
    AF = mybir.ActivationFunctionType
    ALU = mybir.AluOpType

    lch = _halves(L)            # [(0,768),(768,768),(1536,384)] for L=1920

    nc = bacc.Bacc("TRN2", target_bir_lowering=False, debug=False)
    xT = nc.declare_dram_parameter("xT", [C, NKEY], bf16, isOutput=False)
    qT = nc.declare_dram_parameter("qT", [C, L], bf16, isOutput=False)
    wqT = nc.declare_dram_parameter("wqT", [C, C], bf16, isOutput=False)
    wkT = nc.declare_dram_parameter("wkT", [C, C], bf16, isOutput=False)
    wvT = nc.declare_dram_parameter("wvT", [C, C], bf16, isOutput=False)
    wpT = nc.declare_dram_parameter("wpT", [C, C], bf16, isOutput=False)
    biasP = nc.declare_dram_parameter("biasP", [128, CT], f32, isOutput=False)
    outT = nc.declare_dram_parameter("out", [C, L], f32, isOutput=True)

    with ExitStack() as ctx:
        tc = ctx.enter_context(tile.TileContext(nc))
        pers = ctx.enter_context(tc.tile_pool(name="pers", bufs=1))
        # scores psum: 2 x [128,1024] f32 = 4 banks
        psS = ctx.enter_context(tc.tile_pool(name="psS", bufs=2, space="PSUM"))
        # small psum: attnV poz / Z tile / yproj / prologue proj = 4 banks
        psZ = ctx.enter_context(tc.tile_pool(name="psZ", bufs=4, space="PSUM"))
        ptp = ctx.enter_context(tc.tile_pool(name="ptp", bufs=18))
        work = ctx.enter_context(tc.tile_pool(name="work", bufs=3))

        # ---- persistent inputs -------------------------------------------
        xt_sb = [pers.tile([128, NKEY], bf16, tag=f"xt{i}", name=f"xt{i}") for i in range(CT)]
        qt_sb = [pers.tile([128, L], bf16, tag=f"qt{i}", name=f"qt{i}") for i in range(CT)]
        wq_sb = [pers.tile([128, C], bf16, tag=f"wq{i}", name=f"wq{i}") for i in range(CT)]
        wk_sb = [pers.tile([128, C], bf16, tag=f"wk{i}", name=f"wk{i}") for i in range(CT)]
        wv_sb = [pers.tile([128, C], bf16, tag=f"wv{i}", name=f"wv{i}") for i in range(CT)]
        wp_sb = [pers.tile([128, C], bf16, tag=f"wp{i}", name=f"wp{i}") for i in range(CT)]
        for i in range(CT):
            sl = slice(128 * i, 128 * (i + 1))
            nc.sync.dma_start(xt_sb[i][:], xT[sl, :])
            nc.sync.dma_start(wk_sb[i][:], wkT[sl, :])
            nc.sync.dma_start(wv_sb[i][:], wvT[sl, :])
        for i in range(CT):
            sl = slice(128 * i, 128 * (i + 1))
            nc.sync.dma_start(wq_sb[i][:], wqT[sl, :])
            nc.sync.dma_start(qt_sb[i][:], qT[sl, :])
            nc.sync.dma_start(wp_sb[i][:], wpT[sl, :])
        bias_sb = pers.tile([128, CT], f32, tag="bias")
        nc.sync.dma_start(bias_sb[:], biasP[:])

        ones_sb = pers.tile([128, 128], bf16, tag="ones")
        nc.vector.memset(ones_sb[:], 1.0)
        # 1/32-valued stationary for the 4-slot Z reduction (32-row blocks);
        # the later all-128-row ones broadcast then sums 32x4 replicas to Z
        c32_sb = pers.tile([128, 32], bf16, tag="c32")
        nc.vector.memset(c32_sb[:], 1.0 / 32.0)

        # ---- projections --------------------------------------------------
        kt_sb = [pers.tile([128, NKEY], bf16, tag=f"kt{i}", name=f"kt{i}") for i in range(HPAIRS)]
        qht_sb = [pers.tile([128, L], bf16, tag=f"qht{i}", name=f"qht{i}") for i in range(HPAIRS)]
        v_sb = [pers.tile([128, H * D], bf16, tag=f"va{i}", name=f"va{i}") for i in range(NT)]

        def proj_kt(jt):
            for nch in range(NKEY // 512):
                ps = psZ.tile([128, 512], f32, tag="psZ")
                for ct in range(CT):
                    nc.tensor.matmul(
                        ps[:, 0:512],
                        lhsT=wk_sb[ct][:, 128 * jt:128 * (jt + 1)],
                        rhs=xt_sb[ct][:, 512 * nch:512 * (nch + 1)],
                        start=(ct == 0), stop=(ct == CT - 1))
                nc.vector.tensor_copy(
                    kt_sb[jt][:, 512 * nch:512 * (nch + 1)], ps[:, 0:512])

        def proj_qht(jt):
            for qch in range((L + 511) // 512):
                qs, qn = 512 * qch, min(512, L - 512 * qch)
                ps = psZ.tile([128, 512], f32, tag="psZ")
                for ct in range(CT):
                    nc.tensor.matmul(
                        ps[:, 0:qn],
                        lhsT=wq_sb[ct][:, 128 * jt:128 * (jt + 1)],
                        rhs=qt_sb[ct][:, qs:qs + qn],
                        start=(ct == 0), stop=(ct == CT - 1))
                nc.vector.tensor_copy(qht_sb[jt][:, qs:qs + qn], ps[:, 0:qn])

        def proj_v(nt):
            ps = psZ.tile([128, 512], f32, tag="psZ")
            for ct in range(CT):
                nc.tensor.matmul(
                    ps[:, 0:512],
                    lhsT=xt_sb[ct][:, 128 * nt:128 * (nt + 1)],
                    rhs=wv_sb[ct][:, 0:C],
                    start=(ct == 0), stop=(ct == CT - 1))
            nc.vector.tensor_copy(v_sb[nt][:], ps[:, 0:512])

        # ---- attention: one (head-pair, l-half) group --------------------
        def attn_scores(hp, lc_i):
            """scores + exp for all 16 key tiles of (hp, half lc_i).
            pt layout [128, 1024]: h1 cols 0:512, h2 cols 512:1024."""
            lcs, lcn = lch[lc_i]
            pts = []
            for nt in range(NT):
                nsl = slice(128 * nt, 128 * (nt + 1))
                ps = psS.tile([128, 1024], f32, tag="psS")
                for hh in range(2):
                    nc.tensor.matmul(
                        ps[:, 512 * hh:512 * hh + lcn],
                        lhsT=kt_sb[hp][64 * hh:64 * hh + 64, nsl],
                        rhs=qht_sb[hp][64 * hh:64 * hh + 64, lcs:lcs + lcn],
                        start=True, stop=True,
                        tile_position=(64 * hh, 0))
                pt = ptp.tile([128, 1024], bf16, tag="pt")
                ps3 = ps[:].rearrange("p (b x) -> p b x", b=2)
                pt3 = pt[:].rearrange("p (b x) -> p b x", b=2)
                if nt < NT_DVE:
                    pti = pt[:].bitcast(i16).rearrange("p (b x) -> p b x", b=2)
                    nc.vector.tensor_scalar(
                        pti[:, :, 0:lcn], ps3[:, :, 0:lcn],
                        SCH_A, SCH_B, ALU.mult, ALU.add)
                else:
                    nc.scalar.activation(pt3[:, :, 0:lcn], ps3[:, :, 0:lcn],
                                         AF.Exp, scale=SCALE)
                pts.append(pt)
            return pts

        def attn_v(hp, lc_i, pts):
            """attnV (col-tiled pairs) + PE Z-reduction + normalize for both
            heads of hp over l-chunk lc_i. Returns onrm [128, lcn]."""
            lcs, lcn = lch[lc_i]
            onrm = work.tile([128, 512], bf16, tag=f"onrm{hp}", name=f"onrm{hp}")
            for hh in range(2):
                h = 2 * hp + hh
                hoff = hh * 512
                poz = psZ.tile([128, 512], f32, tag="psZ")
                for nt in range(NT):
                    half = nt % 2
                    nc.tensor.matmul(
                        poz[64 * half:64 * half + 64, 0:lcn],
                        lhsT=v_sb[nt][:, 64 * h:64 * h + 64],
                        rhs=pts[nt][:, hoff:hoff + lcn],
                        start=(nt < 2), stop=(nt >= NT - 2),
                        tile_position=(0, 64 * half),
                        skip_group_check=True)
                # Z on PE: 4-way col-tiled reductions into 32-row blocks
                zt = psZ.tile([128, 512], f32, tag="psZ")
                for nt in range(NT):
                    slot = nt % 4
                    nc.tensor.matmul(
                        zt[32 * slot:32 * slot + 32, 0:lcn],
                        lhsT=c32_sb[:, 0:32],
                        rhs=pts[nt][:, hoff:hoff + lcn],
                        start=(nt < 4), stop=(nt >= NT - 4),
                        tile_position=(0, 32 * slot),
                        skip_group_check=True)
                # partition-aligned copy to SBUF, then all-row ones broadcast
                zsb = work.tile([128, 512], bf16, tag="zsb")
                nc.vector.tensor_copy(zsb[:, 0:lcn], zt[:, 0:lcn])
                nc.tensor.matmul(zt[:, 0:lcn], lhsT=ones_sb[:, 0:128],
                                 rhs=zsb[:, 0:lcn], start=True, stop=True)
                bz = work.tile([128, 512], f32, tag="bz")
                nc.vector.reciprocal_approx_fast(bz[:, 0:lcn], zt[:, 0:lcn])
                o1 = work.tile([64, 512], f32, tag="o1")
                nc.vector.tensor_copy(o1[:, 0:lcn], poz[64:128, 0:lcn])
                osum = work.tile([64, 512], f32, tag="osum")
                nc.vector.tensor_tensor(osum[:, 0:lcn], poz[0:64, 0:lcn],
                                        o1[:, 0:lcn], ALU.add)
                nc.vector.tensor_tensor(
                    onrm[64 * hh:64 * hh + 64, 0:lcn],
                    osum[:, 0:lcn], bz[0:64, 0:lcn], ALU.mult)
            return onrm

        def proj_out(lc_i, onrms):
            lcs, lcn = lch[lc_i]
            for jt in range(CT):
                py = psZ.tile([128, 512], f32, tag="psZ")
                ys = work.tile([128, 512], f32, tag="ys")
                for hp in range(HPAIRS):
                    nc.tensor.matmul(
                        py[:, 0:lcn],
                        lhsT=wp_sb[hp][:, 128 * jt:128 * (jt + 1)],
                        rhs=onrms[hp][:, 0:lcn],
                        start=(hp == 0), stop=(hp == HPAIRS - 1))
                nc.vector.tensor_scalar(
                    ys[:, 0:lcn], py[:, 0:lcn],
                    bias_sb[:, jt:jt + 1], None, ALU.add)
                nc.sync.dma_start(
                    outT[128 * jt:128 * (jt + 1), lcs:lcs + lcn],
                    ys[:, 0:lcn])

        # ---- emission order ----------------------------------------------
        proj_kt(0)
        proj_qht(0)
        proj_kt(1)
        proj_qht(1)
        with tc.high_priority(offset=-(10 ** 6)):
            for nt in range(NT):
                proj_v(nt)
            for jt in range(2, HPAIRS):
                proj_kt(jt)
                proj_qht(jt)
        pending = None
        for lc_i in range(len(lch)):
            onrms = []
            for hp in range(HPAIRS):
                pts = attn_scores(hp, lc_i)
                onrms.append(attn_v(hp, lc_i, pts))
                if hp == 0 and pending is not None:
                    proj_out(*pending)
                    pending = None
            pending = (lc_i, onrms)
        proj_out(*pending)

    nc.compile()
    _BUILD_CACHE[key] = nc
    return nc


def kernel(x, q, Wq, Wkv, Wproj, bproj, q_lengths, max_q_len):
    import ml_dtypes
    from concourse.bass_utils import run_bass_kernel_spmd

    bf16 = ml_dtypes.bfloat16
    x = np.asarray(x, np.float32)
    q = np.asarray(q, np.float32)
    Wq = np.asarray(Wq, np.float32)
    Wkv = np.asarray(Wkv, np.float32)
    Wproj = np.asarray(Wproj, np.float32)
    bproj = np.asarray(bproj, np.float32)
    q_lengths = np.asarray(q_lengths, np.int64)
    assert x.shape[0] == NCORES == B

    L = int(((q_lengths.max() + 127) // 128) * 128)
    nc = _build(L)

    offs = np.concatenate([[0], np.cumsum(q_lengths)])
    wqT = np.ascontiguousarray(Wq.T).astype(bf16)
    wkT = np.ascontiguousarray(Wkv[:C].T).astype(bf16)
    wvT = np.ascontiguousarray(Wkv[C:].T).astype(bf16)
    wpT = np.ascontiguousarray(Wproj.T).astype(bf16)
    biasP = np.ascontiguousarray(bproj.reshape(CT, 128).T).astype(np.float32)

    in_maps = []
    for b in range(B):
        Lb = int(q_lengths[b])
        qseg = q[offs[b]:offs[b] + Lb]
        qTp = np.zeros((C, L), bf16)
        qTp[:, :Lb] = qseg.T.astype(bf16)
        in_maps.append({
            "xT": np.ascontiguousarray(x[b].T).astype(bf16),
            "qT": qTp,
            "wqT": wqT, "wkT": wkT, "wvT": wvT, "wpT": wpT,
            "biasP": biasP,
        })

    trace = os.environ.get("KERNEL_TRACE", "") == "1"
    if trace:
        try:
            import sys
            import types
            import antenv
            if "antenv.axon_hooks" not in sys.modules:
                from trn_agent_boot.trn_boot import _ntff_profile_via_ctypes
                hook = _ntff_profile_via_ctypes("/opt/axon/libaxon_pjrt.so")
                mod = types.ModuleType("antenv.axon_hooks")
                mod.get_axon_ntff_profile_hook = lambda: hook
                sys.modules["antenv.axon_hooks"] = mod
                antenv.axon_hooks = mod
        except Exception as e:
            print(f"ntff hook setup failed: {e}")
            trace = False
    res = run_bass_kernel_spmd(nc, in_maps, core_ids=list(range(NCORES)),
                               trace=trace)
    if trace and res.exec_time_ns is not None:
        print(f"HW exec time: {res.exec_time_ns} ns")
        if res.instructions_and_trace:
            print(f"trace: {res.instructions_and_trace[1]}")

    out = np.empty((int(offs[-1]), C), np.float32)
    for b in range(B):
        Lb = int(q_lengths[b])
        out[offs[b]:offs[b] + Lb] = res.results[b]["out"][:, :Lb].T
    return out


# revision 21
# speedup vs baseline: 1.7849x; 1.3945x over previous
"""Trainium2 Bass kernel: varlen batched cross-attention (sparse_attention).

Math (per reference):
  qh = q @ Wq.T           [Tq, H, D]
  k,v = split(x @ Wkv.T)  [B, N, H, D]
  per batch b: queries of segment b attend over batch b's N keys
  out = softmax(qh k^T / sqrt(D)) v  -> [Tq, C] @ Wproj.T + bproj

Sharding: batch-parallel over 8 cores (core b owns batch b), zero
collectives. Host pre-transposes all operands so every device matmul
contracts over the partition axis. All queries padded to a uniform L
(multiple of 128) so one NEFF serves all cores.

Device layout (per core):
  xT [C, N], qT [C, L] bf16  (feature-major)
  K^T computed as head-pair tiles kt[hp] [128, N]  (d on partitions)
  S^T = K^T_h . qhT_h  per 128-key tile -> exp on ScalarE (scale fused)
  O^T + Z via V-augmented (ones col) matmuls, col-paired heads
  normalize with 1/Z broadcast through a tiny PE matmul
  y^T = Wproj^T . O^T + bias -> DMA out [C, L] f32; host transposes back
"""

import os
import numpy as np

B, NKEY, C, H, D = 8, 2048, 512, 8, 64
NCORES = 8
CT = C // 128          # 4 c-tiles
NT = NKEY // 128       # 16 key tiles
HPAIRS = H // 2        # 4 head pairs
SCALE = float(D) ** -0.5
NT_DVE = 3             # key tiles exp'd on VectorE (Schraudolph)
LOG2E = 1.4426950408889634
SCH_A = 128.0 * LOG2E * SCALE
SCH_B = 127.0 * 128.0 - 7.0

_BUILD_CACHE = {}


def _lchunks(L):
    out = []
    off = 0
    while off < L:
        sz = min(512, L - off)
        out.append((off, sz))
        off += sz
    return out


def _build(L, dbg=False):
    key = (L, dbg)
    if key in _BUILD_CACHE:
        return _BUILD_CACHE[key]
    from contextlib import ExitStack
    import concourse.bass as bass
    import concourse.tile as tile
    import concourse.mybir as mybir
    from concourse import bacc

    f32 = mybir.dt.float32
    bf16 = mybir.dt.bfloat16
    i16 = mybir.dt.int16
    AF = mybir.ActivationFunctionType
    ALU = mybir.AluOpType

    lch = _lchunks(L)

    nc = bacc.Bacc("TRN2", target_bir_lowering=False, debug=False)
    xT = nc.declare_dram_parameter("xT", [C, NKEY], bf16, isOutput=False)
    qT = nc.declare_dram_parameter("qT", [C, L], bf16, isOutput=False)
    wqT = nc.declare_dram_parameter("wqT", [C, C], bf16, isOutput=False)
    wkT = nc.declare_dram_parameter("wkT", [C, C], bf16, isOutput=False)
    wvT = nc.declare_dram_parameter("wvT", [C, C], bf16, isOutput=False)
    wpT = nc.declare_dram_parameter("wpT", [C, C], bf16, isOutput=False)
    biasP = nc.declare_dram_parameter("biasP", [128, CT], f32, isOutput=False)
    outT = nc.declare_dram_parameter("out", [C, L], f32, isOutput=True)
    if dbg:
        dbg_kt = nc.declare_dram_parameter("dbg_kt", [128, NKEY], f32, isOutput=True)
        dbg_qht = nc.declare_dram_parameter("dbg_qht", [128, L], f32, isOutput=True)
        dbg_pt = nc.declare_dram_parameter("dbg_pt", [128, 1024], f32, isOutput=True)
        dbg_poz = nc.declare_dram_parameter("dbg_poz", [128, 1024], f32, isOutput=True)
        dbg_onrm = nc.declare_dram_parameter("dbg_onrm", [128, 512], f32, isOutput=True)

    with ExitStack() as ctx:
        tc = ctx.enter_context(tile.TileContext(nc))
        pers = ctx.enter_context(tc.tile_pool(name="pers", bufs=1))
        psS = ctx.enter_context(tc.tile_pool(name="psS", bufs=2, space="PSUM"))
        psOZ = ctx.enter_context(tc.tile_pool(name="psOZ", bufs=2, space="PSUM"))
        psP = ctx.enter_context(tc.tile_pool(name="psP", bufs=2, space="PSUM"))
        ptp = ctx.enter_context(tc.tile_pool(name="ptp", bufs=24))
        work = ctx.enter_context(tc.tile_pool(name="work", bufs=2))

        # ---- persistent inputs -------------------------------------------
        xt_sb = [pers.tile([128, NKEY], bf16, tag=f"xt{i}", name=f"xt{i}") for i in range(CT)]
        qt_sb = [pers.tile([128, L], bf16, tag=f"qt{i}", name=f"qt{i}") for i in range(CT)]
        wq_sb = [pers.tile([128, C], bf16, tag=f"wq{i}", name=f"wq{i}") for i in range(CT)]
        wk_sb = [pers.tile([128, C], bf16, tag=f"wk{i}", name=f"wk{i}") for i in range(CT)]
        wv_sb = [pers.tile([128, C], bf16, tag=f"wv{i}", name=f"wv{i}") for i in range(CT)]
        wp_sb = [pers.tile([128, C], bf16, tag=f"wp{i}", name=f"wp{i}") for i in range(CT)]
        for i in range(CT):
            sl = slice(128 * i, 128 * (i + 1))
            nc.sync.dma_start(xt_sb[i][:], xT[sl, :])
            nc.sync.dma_start(wk_sb[i][:], wkT[sl, :])
            nc.sync.dma_start(wv_sb[i][:], wvT[sl, :])
        for i in range(CT):
            sl = slice(128 * i, 128 * (i + 1))
            nc.sync.dma_start(wq_sb[i][:], wqT[sl, :])
            nc.sync.dma_start(qt_sb[i][:], qT[sl, :])
            nc.sync.dma_start(wp_sb[i][:], wpT[sl, :])
        bias_sb = pers.tile([128, CT], f32, tag="bias")
        nc.sync.dma_start(bias_sb[:], biasP[:])

        # all-ones matrix: matmul(lhsT=ones, rhs=zacc) broadcasts the
        # partition-colsum of zacc to every output partition in one shot
        ones_sb = pers.tile([128, 128], bf16, tag="ones")
        nc.vector.memset(ones_sb[:], 1.0)

        # ---- projections --------------------------------------------------
        kt_sb = [pers.tile([128, NKEY], bf16, tag=f"kt{i}", name=f"kt{i}") for i in range(HPAIRS)]
        qht_sb = [pers.tile([128, L], bf16, tag=f"qht{i}", name=f"qht{i}") for i in range(HPAIRS)]
        vaug_sb = [pers.tile([128, H * (D + 1)], bf16, tag=f"va{i}", name=f"va{i}") for i in range(NT)]

        def proj_kt(jt):
            for nch in range(NKEY // 512):
                ps = psP.tile([128, 512], f32, tag="psP")
                for ct in range(CT):
                    nc.tensor.matmul(
                        ps[:, 0:512],
                        lhsT=wk_sb[ct][:, 128 * jt:128 * (jt + 1)],
                        rhs=xt_sb[ct][:, 512 * nch:512 * (nch + 1)],
                        start=(ct == 0), stop=(ct == CT - 1))
                nc.vector.tensor_copy(
                    kt_sb[jt][:, 512 * nch:512 * (nch + 1)], ps[:, 0:512])

        def proj_qht(jt):
            for (lcs, lcn) in lch:
                ps = psP.tile([128, 512], f32, tag="psP")
                for ct in range(CT):
                    nc.tensor.matmul(
                        ps[:, 0:lcn],
                        lhsT=wq_sb[ct][:, 128 * jt:128 * (jt + 1)],
                        rhs=qt_sb[ct][:, lcs:lcs + lcn],
                        start=(ct == 0), stop=(ct == CT - 1))
                nc.vector.tensor_copy(qht_sb[jt][:, lcs:lcs + lcn], ps[:, 0:lcn])

        def proj_v(nt):
            ps = psP.tile([128, 512], f32, tag="psP")
            for ct in range(CT):
                nc.tensor.matmul(
                    ps[:, 0:512],
                    lhsT=xt_sb[ct][:, 128 * nt:128 * (nt + 1)],
                    rhs=wv_sb[ct][:, 0:C],
                    start=(ct == 0), stop=(ct == CT - 1))
            va3 = vaug_sb[nt][:].rearrange("p (h e) -> p h e", h=H)
            ps3 = ps[:, 0:512].rearrange("p (h d) -> p h d", h=H)
            nc.vector.tensor_copy(va3[:, :, 0:D], ps3[:, :, :])
            nc.vector.memset(va3[:, :, D:D + 1], 1.0)

        # ---- attention group: head pair hp, l-chunk lc -------------------
        def attn(lc_i, hp):
            lcs, lcn = lch[lc_i]
            h1, h2 = 2 * hp, 2 * hp + 1
            pts = []
            for nt in range(NT):
                ps = psS.tile([128, 1024], f32, tag="psS")
                nsl = slice(128 * nt, 128 * (nt + 1))
                nc.tensor.matmul(
                    ps[:, 0:lcn],
                    lhsT=kt_sb[hp][0:64, nsl],
                    rhs=qht_sb[hp][0:64, lcs:lcs + lcn],
                    start=True, stop=True, tile_position=(0, 0))
                nc.tensor.matmul(
                    ps[:, 512:512 + lcn],
                    lhsT=kt_sb[hp][64:128, nsl],
                    rhs=qht_sb[hp][64:128, lcs:lcs + lcn],
                    start=True, stop=True, tile_position=(64, 0))
                pt = ptp.tile([128, 1024], bf16, tag="pt")
                ps2 = ps[:, 0:1024].rearrange("p (b x) -> p b x", b=2)
                pt2 = pt[:, 0:1024].rearrange("p (b x) -> p b x", b=2)
                if nt >= NT - NT_DVE:
                    pti = pt[:, 0:1024].bitcast(i16).rearrange(
                        "p (b x) -> p b x", b=2)
                    nc.vector.tensor_scalar(
                        pti[:, :, 0:lcn], ps2[:, :, 0:lcn],
                        SCH_A, SCH_B, ALU.mult, ALU.add)
                else:
                    nc.scalar.activation(pt2[:, :, 0:lcn], ps2[:, :, 0:lcn],
                                         AF.Exp, scale=SCALE)
                pts.append(pt)
                if nt == 1:
                    zacc = work.tile([128, 1024], bf16, tag="zacc")
                    nc.vector.tensor_tensor(zacc[:, :], pts[0][:, 0:1024],
                                            pts[1][:, 0:1024], ALU.add)
                elif nt > 1:
                    nc.vector.tensor_tensor(zacc[:, :], zacc[:, :],
                                            pts[nt][:, 0:1024], ALU.add)
            # O^T accumulated over the 16 key tiles (col-paired heads).
            poz = psOZ.tile([128, 512], f32, tag="psOZ")
            nc.vector.memset(poz[:, :], 0.0)
            for nt in range(NT):
                va3 = vaug_sb[nt][:].rearrange("p (h e) -> p h e", h=H)
                stop = (nt == NT - 1)
                nc.tensor.matmul(
                    poz[0:64, 0:lcn], lhsT=va3[:, h1, 0:D],
                    rhs=pts[nt][:, 0:lcn],
                    start=False, stop=stop, tile_position=(0, 0),
                    skip_group_check=True)
                nc.tensor.matmul(
                    poz[64:128, 0:lcn], lhsT=va3[:, h2, 0:D],
                    rhs=pts[nt][:, 512:512 + lcn],
                    start=False, stop=stop, tile_position=(0, 64),
                    skip_group_check=True)
            # broadcast-sum Z to all partitions: ones^T . zacc
            pbz = psP.tile([128, 512], f32, tag="psP")
            nc.tensor.matmul(pbz[:, 0:lcn], lhsT=ones_sb[:, 0:128],
                             rhs=zacc[:, 0:lcn], start=True, stop=True)
            pbz2 = psP.tile([128, 512], f32, tag="psP")
            nc.tensor.matmul(pbz2[:, 0:lcn], lhsT=ones_sb[:, 0:128],
                             rhs=zacc[:, 512:512 + lcn], start=True, stop=True)
            if dbg and lc_i == 0 and hp == 0:
                dcp2 = work.tile([128, 1024], f32, tag="dcp2")
                nc.vector.tensor_copy(dcp2[:, :], poz[:, 0:1024])
                nc.sync.dma_start(dbg_poz[:, :], dcp2[:, :])
            bz_sb = work.tile([128, 1024], f32, tag="bz")
            nc.vector.reciprocal_approx_fast(bz_sb[:, 0:lcn], pbz[:, 0:lcn])
            nc.vector.reciprocal_approx_fast(bz_sb[:, 512:512 + lcn],
                                             pbz2[:, 0:lcn])
            onrm = work.tile([128, 512], bf16, tag=f"onrm{hp}")
            nc.vector.tensor_tensor(onrm[0:64, 0:lcn], poz[0:64, 0:lcn],
                                    bz_sb[0:64, 0:lcn], ALU.mult)
            nc.vector.tensor_tensor(onrm[64:128, 0:lcn], poz[64:128, 0:lcn],
                                    bz_sb[64:128, 512:512 + lcn], ALU.mult)
            if dbg and lc_i == 0 and hp == 0:
                dcp3 = work.tile([128, 512], f32, tag="dcp3")
                nc.vector.tensor_copy(dcp3[:, :], onrm[:, 0:512])
                nc.sync.dma_start(dbg_onrm[:, :], dcp3[:, :])
            return onrm

        def proj_out(lc_i, onrms):
            lcs, lcn = lch[lc_i]
            for jt in range(CT):
                py = psOZ.tile([128, 512], f32, tag="psOZ")
                ys = work.tile([128, 512], f32, tag="ys")
                for hp in range(HPAIRS):
                    nc.tensor.matmul(
                        py[:, 0:lcn],
                        lhsT=wp_sb[hp][:, 128 * jt:128 * (jt + 1)],
                        rhs=onrms[hp][:, 0:lcn],
                        start=(hp == 0), stop=(hp == HPAIRS - 1))
                nc.vector.tensor_scalar(
                    ys[:, 0:lcn], py[:, 0:lcn],
                    bias_sb[:, jt:jt + 1], None, ALU.add)
                nc.sync.dma_start(
                    outT[128 * jt:128 * (jt + 1), lcs:lcs + lcn],
                    ys[:, 0:lcn])

        # ---- emission order (scheduling priority) ------------------------
        proj_kt(0)
        proj_qht(0)
        proj_kt(1)
        proj_qht(1)
        # Remaining projections at background priority: they run in PE gaps
        # of the ACT-bound exp stream instead of serializing up front.
        with tc.high_priority(offset=-(10 ** 6)):
            for nt in range(NT):
                proj_v(nt)
            for jt in range(2, HPAIRS):
                proj_kt(jt)
                proj_qht(jt)
        if dbg:
            dk = work.tile([128, NKEY], f32, tag="dk", bufs=1)
            nc.vector.tensor_copy(dk[:, :], kt_sb[0][:, :])
            nc.sync.dma_start(dbg_kt[:, :], dk[:, :])
            dq = work.tile([128, L], f32, tag="dq", bufs=1)
            nc.vector.tensor_copy(dq[:, :], qht_sb[0][:, :])
            nc.sync.dma_start(dbg_qht[:, :], dq[:, :])
        pending = None
        for lc_i in range(len(lch)):
            onrms = []
            for hp in range(HPAIRS):
                onrms.append(attn(lc_i, hp))
                if hp == 0 and pending is not None:
                    proj_out(*pending)
                    pending = None
            pending = (lc_i, onrms)
        proj_out(*pending)

    nc.compile()
    _BUILD_CACHE[key] = nc
    return nc


def kernel(x, q, Wq, Wkv, Wproj, bproj, q_lengths, max_q_len):
    import ml_dtypes
    from concourse.bass_utils import run_bass_kernel_spmd

    bf16 = ml_dtypes.bfloat16
    x = np.asarray(x, np.float32)
    q = np.asarray(q, np.float32)
    Wq = np.asarray(Wq, np.float32)
    Wkv = np.asarray(Wkv, np.float32)
    Wproj = np.asarray(Wproj, np.float32)
    bproj = np.asarray(bproj, np.float32)
    q_lengths = np.asarray(q_lengths, np.int64)
    assert x.shape[0] == NCORES == B

    L = int(((q_lengths.max() + 127) // 128) * 128)
    nc = _build(L)

    offs = np.concatenate([[0], np.cumsum(q_lengths)])
    wqT = np.ascontiguousarray(Wq.T).astype(bf16)
    wkT = np.ascontiguousarray(Wkv[:C].T).astype(bf16)
    wvT = np.ascontiguousarray(Wkv[C:].T).astype(bf16)
    wpT = np.ascontiguousarray(Wproj.T).astype(bf16)
    biasP = np.ascontiguousarray(bproj.reshape(CT, 128).T).astype(np.float32)

    in_maps = []
    for b in range(B):
        Lb = int(q_lengths[b])
        qseg = q[offs[b]:offs[b] + Lb]
        qTp = np.zeros((C, L), bf16)
        qTp[:, :Lb] = qseg.T.astype(bf16)
        in_maps.append({
            "xT": np.ascontiguousarray(x[b].T).astype(bf16),
            "qT": qTp,
            "wqT": wqT, "wkT": wkT, "wvT": wvT, "wpT": wpT,
            "biasP": biasP,
        })

    trace = os.environ.get("KERNEL_TRACE", "") == "1"
    if trace:
        try:
            import sys
            import types
            import antenv
            if "antenv.axon_hooks" not in sys.modules:
                from trn_agent_boot.trn_boot import _ntff_profile_via_ctypes
                hook = _ntff_profile_via_ctypes("/opt/axon/libaxon_pjrt.so")
                mod = types.ModuleType("antenv.axon_hooks")
                mod.get_axon_ntff_profile_hook = lambda: hook
                sys.modules["antenv.axon_hooks"] = mod
                antenv.axon_hooks = mod
        except Exception as e:
            print(f"ntff hook setup failed: {e}")
            trace = False
    res = run_bass_kernel_spmd(nc, in_maps, core_ids=list(range(NCORES)),
                               trace=trace)
    if trace and res.exec_time_ns is not None:
        print(f"HW exec time: {res.exec_time_ns} ns")
        if res.instructions_and_trace:
            print(f"trace: {res.instructions_and_trace[1]}")

    out = np.empty((int(offs[-1]), C), np.float32)
    for b in range(B):
        Lb = int(q_lengths[b])
        out[offs[b]:offs[b] + Lb] = res.results[b]["out"][:, :Lb].T
    return out



# revision 22
# speedup vs baseline: 2.0329x; 1.1389x over previous
"""Trainium2 Bass kernel: varlen batched cross-attention (sparse_attention).

Math (per reference):
  qh = q @ Wq.T           [Tq, H, D]
  k,v = split(x @ Wkv.T)  [B, N, H, D]
  per batch b: queries of segment b attend over batch b's N keys
  out = softmax(qh k^T / sqrt(D)) v  -> [Tq, C] @ Wproj.T + bproj

Sharding: batch-parallel over 8 cores (core b owns batch b), zero
collectives. Host pre-transposes all operands so every device matmul
contracts over the partition axis. All queries padded to a uniform L
(multiple of 128) so one NEFF serves all cores.

Device layout (per core):
  xT [C, N], qT [C, L] bf16  (feature-major)
  K^T computed as head-pair tiles kt[hp] [128, N]  (d on partitions)
  S^T = K^T_h . qhT_h  per 128-key tile -> exp on ScalarE (scale fused)
  O^T + Z via V-augmented (ones col) matmuls, col-paired heads
  normalize with 1/Z broadcast through a tiny PE matmul
  y^T = Wproj^T . O^T + bias -> DMA out [C, L] f32; host transposes back
"""

import os
import numpy as np

B, NKEY, C, H, D = 8, 2048, 512, 8, 64
NCORES = 8
CT = C // 128          # 4 c-tiles
NT = NKEY // 128       # 16 key tiles
HPAIRS = H // 2        # 4 head pairs
SCALE = float(D) ** -0.5

_BUILD_CACHE = {}


def _lchunks(L):
    out = []
    off = 0
    while off < L:
        sz = min(512, L - off)
        out.append((off, sz))
        off += sz
    return out


def _build(L, dbg=False):
    key = (L, dbg)
    if key in _BUILD_CACHE:
        return _BUILD_CACHE[key]
    from contextlib import ExitStack
    import concourse.bass as bass
    import concourse.tile as tile
    import concourse.mybir as mybir
    from concourse import bacc

    f32 = mybir.dt.float32
    bf16 = mybir.dt.bfloat16
    AF = mybir.ActivationFunctionType
    ALU = mybir.AluOpType

    lch = _lchunks(L)

    nc = bacc.Bacc("TRN2", target_bir_lowering=False, debug=False)
    xT = nc.declare_dram_parameter("xT", [C, NKEY], bf16, isOutput=False)
    qT = nc.declare_dram_parameter("qT", [C, L], bf16, isOutput=False)
    wqT = nc.declare_dram_parameter("wqT", [C, C], bf16, isOutput=False)
    wkT = nc.declare_dram_parameter("wkT", [C, C], bf16, isOutput=False)
    wvT = nc.declare_dram_parameter("wvT", [C, C], bf16, isOutput=False)
    wpT = nc.declare_dram_parameter("wpT", [C, C], bf16, isOutput=False)
    biasP = nc.declare_dram_parameter("biasP", [128, CT], f32, isOutput=False)
    outT = nc.declare_dram_parameter("out", [C, L], f32, isOutput=True)
    if dbg:
        dbg_kt = nc.declare_dram_parameter("dbg_kt", [128, NKEY], f32, isOutput=True)
        dbg_qht = nc.declare_dram_parameter("dbg_qht", [128, L], f32, isOutput=True)
        dbg_pt = nc.declare_dram_parameter("dbg_pt", [128, 1024], f32, isOutput=True)
        dbg_poz = nc.declare_dram_parameter("dbg_poz", [128, 1024], f32, isOutput=True)
        dbg_onrm = nc.declare_dram_parameter("dbg_onrm", [128, 512], f32, isOutput=True)

    with ExitStack() as ctx:
        tc = ctx.enter_context(tile.TileContext(nc))
        pers = ctx.enter_context(tc.tile_pool(name="pers", bufs=1))
        psS = ctx.enter_context(tc.tile_pool(name="psS", bufs=2, space="PSUM"))
        psOZ = ctx.enter_context(tc.tile_pool(name="psOZ", bufs=2, space="PSUM"))
        psP = ctx.enter_context(tc.tile_pool(name="psP", bufs=2, space="PSUM"))
        ptp = ctx.enter_context(tc.tile_pool(name="ptp", bufs=24))
        work = ctx.enter_context(tc.tile_pool(name="work", bufs=2))

        # ---- persistent inputs -------------------------------------------
        xt_sb = [pers.tile([128, NKEY], bf16, tag=f"xt{i}", name=f"xt{i}") for i in range(CT)]
        qt_sb = [pers.tile([128, L], bf16, tag=f"qt{i}", name=f"qt{i}") for i in range(CT)]
        wq_sb = [pers.tile([128, C], bf16, tag=f"wq{i}", name=f"wq{i}") for i in range(CT)]
        wk_sb = [pers.tile([128, C], bf16, tag=f"wk{i}", name=f"wk{i}") for i in range(CT)]
        wv_sb = [pers.tile([128, C], bf16, tag=f"wv{i}", name=f"wv{i}") for i in range(CT)]
        wp_sb = [pers.tile([128, C], bf16, tag=f"wp{i}", name=f"wp{i}") for i in range(CT)]
        for i in range(CT):
            sl = slice(128 * i, 128 * (i + 1))
            nc.sync.dma_start(xt_sb[i][:], xT[sl, :])
            nc.sync.dma_start(wk_sb[i][:], wkT[sl, :])
            nc.sync.dma_start(wv_sb[i][:], wvT[sl, :])
        for i in range(CT):
            sl = slice(128 * i, 128 * (i + 1))
            nc.sync.dma_start(wq_sb[i][:], wqT[sl, :])
            nc.sync.dma_start(qt_sb[i][:], qT[sl, :])
            nc.sync.dma_start(wp_sb[i][:], wpT[sl, :])
        bias_sb = pers.tile([128, CT], f32, tag="bias")
        nc.sync.dma_start(bias_sb[:], biasP[:])

        # all-ones matrix: matmul(lhsT=ones, rhs=zacc) broadcasts the
        # partition-colsum of zacc to every output partition in one shot
        ones_sb = pers.tile([128, 128], bf16, tag="ones")
        nc.vector.memset(ones_sb[:], 1.0)

        # ---- projections --------------------------------------------------
        kt_sb = [pers.tile([128, NKEY], bf16, tag=f"kt{i}", name=f"kt{i}") for i in range(HPAIRS)]
        qht_sb = [pers.tile([128, L], bf16, tag=f"qht{i}", name=f"qht{i}") for i in range(HPAIRS)]
        vaug_sb = [pers.tile([128, H * (D + 1)], bf16, tag=f"va{i}", name=f"va{i}") for i in range(NT)]

        def proj_kt(jt):
            for nch in range(NKEY // 512):
                ps = psP.tile([128, 512], f32, tag="psP")
                for ct in range(CT):
                    nc.tensor.matmul(
                        ps[:, 0:512],
                        lhsT=wk_sb[ct][:, 128 * jt:128 * (jt + 1)],
                        rhs=xt_sb[ct][:, 512 * nch:512 * (nch + 1)],
                        start=(ct == 0), stop=(ct == CT - 1))
                nc.vector.tensor_copy(
                    kt_sb[jt][:, 512 * nch:512 * (nch + 1)], ps[:, 0:512])

        def proj_qht(jt):
            for (lcs, lcn) in lch:
                ps = psP.tile([128, 512], f32, tag="psP")
                for ct in range(CT):
                    nc.tensor.matmul(
                        ps[:, 0:lcn],
                        lhsT=wq_sb[ct][:, 128 * jt:128 * (jt + 1)],
                        rhs=qt_sb[ct][:, lcs:lcs + lcn],
                        start=(ct == 0), stop=(ct == CT - 1))
                nc.vector.tensor_copy(qht_sb[jt][:, lcs:lcs + lcn], ps[:, 0:lcn])

        def proj_v(nt):
            ps = psP.tile([128, 512], f32, tag="psP")
            for ct in range(CT):
                nc.tensor.matmul(
                    ps[:, 0:512],
                    lhsT=xt_sb[ct][:, 128 * nt:128 * (nt + 1)],
                    rhs=wv_sb[ct][:, 0:C],
                    start=(ct == 0), stop=(ct == CT - 1))
            va3 = vaug_sb[nt][:].rearrange("p (h e) -> p h e", h=H)
            ps3 = ps[:, 0:512].rearrange("p (h d) -> p h d", h=H)
            nc.vector.tensor_copy(va3[:, :, 0:D], ps3[:, :, :])
            nc.vector.memset(va3[:, :, D:D + 1], 1.0)

        # ---- attention group: head pair hp, l-chunk lc -------------------
        def attn(lc_i, hp):
            lcs, lcn = lch[lc_i]
            h1, h2 = 2 * hp, 2 * hp + 1
            pts = []
            for nt in range(NT):
                ps = psS.tile([128, 1024], f32, tag="psS")
                nsl = slice(128 * nt, 128 * (nt + 1))
                nc.tensor.matmul(
                    ps[:, 0:lcn],
                    lhsT=kt_sb[hp][0:64, nsl],
                    rhs=qht_sb[hp][0:64, lcs:lcs + lcn],
                    start=True, stop=True, tile_position=(0, 0))
                nc.tensor.matmul(
                    ps[:, 512:512 + lcn],
                    lhsT=kt_sb[hp][64:128, nsl],
                    rhs=qht_sb[hp][64:128, lcs:lcs + lcn],
                    start=True, stop=True, tile_position=(64, 0))
                pt = ptp.tile([128, 1024], bf16, tag="pt")
                ps2 = ps[:, 0:1024].rearrange("p (b x) -> p b x", b=2)
                pt2 = pt[:, 0:1024].rearrange("p (b x) -> p b x", b=2)
                nc.scalar.activation(pt2[:, :, 0:lcn], ps2[:, :, 0:lcn],
                                     AF.Exp, scale=SCALE)
                pts.append(pt)
                if nt == 1:
                    zacc = work.tile([128, 1024], bf16, tag="zacc")
                    nc.vector.tensor_tensor(zacc[:, :], pts[0][:, 0:1024],
                                            pts[1][:, 0:1024], ALU.add)
                elif nt > 1:
                    nc.vector.tensor_tensor(zacc[:, :], zacc[:, :],
                                            pts[nt][:, 0:1024], ALU.add)
            # O^T accumulated over the 16 key tiles (col-paired heads).
            poz = psOZ.tile([128, 512], f32, tag="psOZ")
            nc.vector.memset(poz[:, :], 0.0)
            for nt in range(NT):
                va3 = vaug_sb[nt][:].rearrange("p (h e) -> p h e", h=H)
                stop = (nt == NT - 1)
                nc.tensor.matmul(
                    poz[0:64, 0:lcn], lhsT=va3[:, h1, 0:D],
                    rhs=pts[nt][:, 0:lcn],
                    start=False, stop=stop, tile_position=(0, 0),
                    skip_group_check=True)
                nc.tensor.matmul(
                    poz[64:128, 0:lcn], lhsT=va3[:, h2, 0:D],
                    rhs=pts[nt][:, 512:512 + lcn],
                    start=False, stop=stop, tile_position=(0, 64),
                    skip_group_check=True)
            # broadcast-sum Z to all partitions: ones^T . zacc
            pbz = psP.tile([128, 512], f32, tag="psP")
            nc.tensor.matmul(pbz[:, 0:lcn], lhsT=ones_sb[:, 0:128],
                             rhs=zacc[:, 0:lcn], start=True, stop=True)
            pbz2 = psP.tile([128, 512], f32, tag="psP")
            nc.tensor.matmul(pbz2[:, 0:lcn], lhsT=ones_sb[:, 0:128],
                             rhs=zacc[:, 512:512 + lcn], start=True, stop=True)
            if dbg and lc_i == 0 and hp == 0:
                dcp2 = work.tile([128, 1024], f32, tag="dcp2")
                nc.vector.tensor_copy(dcp2[:, :], poz[:, 0:1024])
                nc.sync.dma_start(dbg_poz[:, :], dcp2[:, :])
            bz_sb = work.tile([128, 1024], f32, tag="bz")
            nc.vector.reciprocal_approx_fast(bz_sb[:, 0:lcn], pbz[:, 0:lcn])
            nc.vector.reciprocal_approx_fast(bz_sb[:, 512:512 + lcn],
                                             pbz2[:, 0:lcn])
            onrm = work.tile([128, 512], bf16, tag=f"onrm{hp}")
            nc.vector.tensor_tensor(onrm[0:64, 0:lcn], poz[0:64, 0:lcn],
                                    bz_sb[0:64, 0:lcn], ALU.mult)
            nc.vector.tensor_tensor(onrm[64:128, 0:lcn], poz[64:128, 0:lcn],
                                    bz_sb[64:128, 512:512 + lcn], ALU.mult)
            if dbg and lc_i == 0 and hp == 0:
                dcp3 = work.tile([128, 512], f32, tag="dcp3")
                nc.vector.tensor_copy(dcp3[:, :], onrm[:, 0:512])
                nc.sync.dma_start(dbg_onrm[:, :], dcp3[:, :])
            return onrm

        def proj_out(lc_i, onrms):
            lcs, lcn = lch[lc_i]
            for jt in range(CT):
                py = psOZ.tile([128, 512], f32, tag="psOZ")
                ys = work.tile([128, 512], f32, tag="ys")
                for hp in range(HPAIRS):
                    nc.tensor.matmul(
                        py[:, 0:lcn],
                        lhsT=wp_sb[hp][:, 128 * jt:128 * (jt + 1)],
                        rhs=onrms[hp][:, 0:lcn],
                        start=(hp == 0), stop=(hp == HPAIRS - 1))
                nc.vector.tensor_scalar(
                    ys[:, 0:lcn], py[:, 0:lcn],
                    bias_sb[:, jt:jt + 1], None, ALU.add)
                nc.sync.dma_start(
                    outT[128 * jt:128 * (jt + 1), lcs:lcs + lcn],
                    ys[:, 0:lcn])

        # ---- emission order (scheduling priority) ------------------------
        proj_kt(0)
        proj_qht(0)
        proj_kt(1)
        proj_qht(1)
        # Remaining projections at background priority: they run in PE gaps
        # of the ACT-bound exp stream instead of serializing up front.
        with tc.high_priority(offset=-(10 ** 6)):
            for nt in range(NT):
                proj_v(nt)
            for jt in range(2, HPAIRS):
                proj_kt(jt)
                proj_qht(jt)
        if dbg:
            dk = work.tile([128, NKEY], f32, tag="dk", bufs=1)
            nc.vector.tensor_copy(dk[:, :], kt_sb[0][:, :])
            nc.sync.dma_start(dbg_kt[:, :], dk[:, :])
            dq = work.tile([128, L], f32, tag="dq", bufs=1)
            nc.vector.tensor_copy(dq[:, :], qht_sb[0][:, :])
            nc.sync.dma_start(dbg_qht[:, :], dq[:, :])
        pending = None
        for lc_i in range(len(lch)):
            onrms = []
            for hp in range(HPAIRS):
                onrms.append(attn(lc_i, hp))
                if hp == 0 and pending is not None:
                    proj_out(*pending)
                    pending = None
            pending = (lc_i, onrms)
        proj_out(*pending)

    nc.compile()
    _BUILD_CACHE[key] = nc
    return nc


def kernel(x, q, Wq, Wkv, Wproj, bproj, q_lengths, max_q_len):
    import ml_dtypes
    from concourse.bass_utils import run_bass_kernel_spmd

    bf16 = ml_dtypes.bfloat16
    x = np.asarray(x, np.float32)
    q = np.asarray(q, np.float32)
    Wq = np.asarray(Wq, np.float32)
    Wkv = np.asarray(Wkv, np.float32)
    Wproj = np.asarray(Wproj, np.float32)
    bproj = np.asarray(bproj, np.float32)
    q_lengths = np.asarray(q_lengths, np.int64)
    assert x.shape[0] == NCORES == B

    L = int(((q_lengths.max() + 127) // 128) * 128)
    nc = _build(L)

    offs = np.concatenate([[0], np.cumsum(q_lengths)])
    wqT = np.ascontiguousarray(Wq.T).astype(bf16)
    wkT = np.ascontiguousarray(Wkv[:C].T).astype(bf16)
    wvT = np.ascontiguousarray(Wkv[C:].T).astype(bf16)
    wpT = np.ascontiguousarray(Wproj.T).astype(bf16)
    biasP = np.ascontiguousarray(bproj.reshape(CT, 128).T).astype(np.float32)

    in_maps = []
    for b in range(B):
        Lb = int(q_lengths[b])
        qseg = q[offs[b]:offs[b] + Lb]
        qTp = np.zeros((C, L), bf16)
        qTp[:, :Lb] = qseg.T.astype(bf16)
        in_maps.append({
            "xT": np.ascontiguousarray(x[b].T).astype(bf16),
            "qT": qTp,
            "wqT": wqT, "wkT": wkT, "wvT": wvT, "wpT": wpT,
            "biasP": biasP,
        })

    trace = os.environ.get("KERNEL_TRACE", "") == "1"
    if trace:
        try:
            import sys
            import types
            import antenv
            if "antenv.axon_hooks" not in sys.modules:
                from trn_agent_boot.trn_boot import _ntff_profile_via_ctypes
                hook = _ntff_profile_via_ctypes("/opt/axon/libaxon_pjrt.so")
                mod = types.ModuleType("antenv.axon_hooks")
                mod.get_axon_ntff_profile_hook = lambda: hook
                sys.modules["antenv.axon_hooks"] = mod
                antenv.axon_hooks = mod
        except Exception as e:
            print(f"ntff hook setup failed: {e}")
            trace = False
    res = run_bass_kernel_spmd(nc, in_maps, core_ids=list(range(NCORES)),
                               trace=trace)
    if trace and res.exec_time_ns is not None:
        print(f"HW exec time: {res.exec_time_ns} ns")
        if res.instructions_and_trace:
            print(f"trace: {res.instructions_and_trace[1]}")

    out = np.empty((int(offs[-1]), C), np.float32)
    for b in range(B):
        Lb = int(q_lengths[b])
        out[offs[b]:offs[b] + Lb] = res.results[b]["out"][:, :Lb].T
    return out



# revision 23
# speedup vs baseline: 2.0571x; 1.0119x over previous
"""Trainium2 Bass kernel: varlen batched cross-attention (sparse_attention).

Math (per reference):
  qh = q @ Wq.T           [Tq, H, D]
  k,v = split(x @ Wkv.T)  [B, N, H, D]
  per batch b: queries of segment b attend over batch b's N keys
  out = softmax(qh k^T / sqrt(D)) v  -> [Tq, C] @ Wproj.T + bproj

Sharding: batch-parallel over 8 cores (core b owns batch b), zero
collectives. Host pre-transposes all operands so every device matmul
contracts over the partition axis. All queries padded to a uniform L
(multiple of 128) so one NEFF serves all cores.

Device layout (per core):
  xT [C, N], qT [C, L] bf16  (feature-major)
  K^T computed as head-pair tiles kt[hp] [128, N]  (d on partitions)
  S^T = K^T_h . qhT_h  per 128-key tile -> exp on ScalarE (scale fused)
  O^T + Z via V-augmented (ones col) matmuls, col-paired heads
  normalize with 1/Z broadcast through a tiny PE matmul
  y^T = Wproj^T . O^T + bias -> DMA out [C, L] f32; host transposes back
"""

import os
import numpy as np

B, NKEY, C, H, D = 8, 2048, 512, 8, 64
NCORES = 8
CT = C // 128          # 4 c-tiles
NT = NKEY // 128       # 16 key tiles
HPAIRS = H // 2        # 4 head pairs
SCALE = float(D) ** -0.5

_BUILD_CACHE = {}


def _lchunks(L):
    out = []
    off = 0
    while off < L:
        sz = min(512, L - off)
        out.append((off, sz))
        off += sz
    return out


def _build(L, dbg=False):
    key = (L, dbg)
    if key in _BUILD_CACHE:
        return _BUILD_CACHE[key]
    from contextlib import ExitStack
    import concourse.bass as bass
    import concourse.tile as tile
    import concourse.mybir as mybir
    from concourse import bacc

    f32 = mybir.dt.float32
    bf16 = mybir.dt.bfloat16
    AF = mybir.ActivationFunctionType
    ALU = mybir.AluOpType

    lch = _lchunks(L)

    nc = bacc.Bacc("TRN2", target_bir_lowering=False, debug=False)
    xT = nc.declare_dram_parameter("xT", [C, NKEY], bf16, isOutput=False)
    qT = nc.declare_dram_parameter("qT", [C, L], bf16, isOutput=False)
    wqT = nc.declare_dram_parameter("wqT", [C, C], bf16, isOutput=False)
    wkT = nc.declare_dram_parameter("wkT", [C, C], bf16, isOutput=False)
    wvT = nc.declare_dram_parameter("wvT", [C, C], bf16, isOutput=False)
    wpT = nc.declare_dram_parameter("wpT", [C, C], bf16, isOutput=False)
    biasP = nc.declare_dram_parameter("biasP", [128, CT], f32, isOutput=False)
    outT = nc.declare_dram_parameter("out", [C, L], f32, isOutput=True)
    if dbg:
        dbg_kt = nc.declare_dram_parameter("dbg_kt", [128, NKEY], f32, isOutput=True)
        dbg_qht = nc.declare_dram_parameter("dbg_qht", [128, L], f32, isOutput=True)
        dbg_pt = nc.declare_dram_parameter("dbg_pt", [128, 1024], f32, isOutput=True)
        dbg_poz = nc.declare_dram_parameter("dbg_poz", [128, 1024], f32, isOutput=True)
        dbg_onrm = nc.declare_dram_parameter("dbg_onrm", [128, 512], f32, isOutput=True)

    with ExitStack() as ctx:
        tc = ctx.enter_context(tile.TileContext(nc))
        pers = ctx.enter_context(tc.tile_pool(name="pers", bufs=1))
        psS = ctx.enter_context(tc.tile_pool(name="psS", bufs=2, space="PSUM"))
        psOZ = ctx.enter_context(tc.tile_pool(name="psOZ", bufs=2, space="PSUM"))
        psP = ctx.enter_context(tc.tile_pool(name="psP", bufs=2, space="PSUM"))
        ptp = ctx.enter_context(tc.tile_pool(name="ptp", bufs=30))
        work = ctx.enter_context(tc.tile_pool(name="work", bufs=3))

        # ---- persistent inputs -------------------------------------------
        xt_sb = [pers.tile([128, NKEY], bf16, tag=f"xt{i}", name=f"xt{i}") for i in range(CT)]
        qt_sb = [pers.tile([128, L], bf16, tag=f"qt{i}", name=f"qt{i}") for i in range(CT)]
        wq_sb = [pers.tile([128, C], bf16, tag=f"wq{i}", name=f"wq{i}") for i in range(CT)]
        wk_sb = [pers.tile([128, C], bf16, tag=f"wk{i}", name=f"wk{i}") for i in range(CT)]
        wv_sb = [pers.tile([128, C], bf16, tag=f"wv{i}", name=f"wv{i}") for i in range(CT)]
        wp_sb = [pers.tile([128, C], bf16, tag=f"wp{i}", name=f"wp{i}") for i in range(CT)]
        for i in range(CT):
            sl = slice(128 * i, 128 * (i + 1))
            nc.sync.dma_start(xt_sb[i][:], xT[sl, :])
            nc.sync.dma_start(wk_sb[i][:], wkT[sl, :])
            nc.sync.dma_start(wv_sb[i][:], wvT[sl, :])
        for i in range(CT):
            sl = slice(128 * i, 128 * (i + 1))
            nc.sync.dma_start(wq_sb[i][:], wqT[sl, :])
            nc.sync.dma_start(qt_sb[i][:], qT[sl, :])
            nc.sync.dma_start(wp_sb[i][:], wpT[sl, :])
        bias_sb = pers.tile([128, CT], f32, tag="bias")
        nc.sync.dma_start(bias_sb[:], biasP[:])

        # all-ones matrix: matmul(lhsT=ones, rhs=zacc) broadcasts the
        # partition-colsum of zacc to every output partition in one shot
        ones_sb = pers.tile([128, 128], bf16, tag="ones")
        nc.vector.memset(ones_sb[:], 1.0)

        # ---- projections --------------------------------------------------
        kt_sb = [pers.tile([128, NKEY], bf16, tag=f"kt{i}", name=f"kt{i}") for i in range(HPAIRS)]
        qht_sb = [pers.tile([128, L], bf16, tag=f"qht{i}", name=f"qht{i}") for i in range(HPAIRS)]
        vaug_sb = [pers.tile([128, H * (D + 1)], bf16, tag=f"va{i}", name=f"va{i}") for i in range(NT)]

        def proj_kt(jt):
            for nch in range(NKEY // 512):
                ps = psP.tile([128, 512], f32, tag="psP")
                for ct in range(CT):
                    nc.tensor.matmul(
                        ps[:, 0:512],
                        lhsT=wk_sb[ct][:, 128 * jt:128 * (jt + 1)],
                        rhs=xt_sb[ct][:, 512 * nch:512 * (nch + 1)],
                        start=(ct == 0), stop=(ct == CT - 1))
                nc.vector.tensor_copy(
                    kt_sb[jt][:, 512 * nch:512 * (nch + 1)], ps[:, 0:512])

        def proj_qht(jt):
            for (lcs, lcn) in lch:
                ps = psP.tile([128, 512], f32, tag="psP")
                for ct in range(CT):
                    nc.tensor.matmul(
                        ps[:, 0:lcn],
                        lhsT=wq_sb[ct][:, 128 * jt:128 * (jt + 1)],
                        rhs=qt_sb[ct][:, lcs:lcs + lcn],
                        start=(ct == 0), stop=(ct == CT - 1))
                nc.vector.tensor_copy(qht_sb[jt][:, lcs:lcs + lcn], ps[:, 0:lcn])

        def proj_v(nt):
            ps = psP.tile([128, 512], f32, tag="psP")
            for ct in range(CT):
                nc.tensor.matmul(
                    ps[:, 0:512],
                    lhsT=xt_sb[ct][:, 128 * nt:128 * (nt + 1)],
                    rhs=wv_sb[ct][:, 0:C],
                    start=(ct == 0), stop=(ct == CT - 1))
            va3 = vaug_sb[nt][:].rearrange("p (h e) -> p h e", h=H)
            ps3 = ps[:, 0:512].rearrange("p (h d) -> p h d", h=H)
            nc.vector.tensor_copy(va3[:, :, 0:D], ps3[:, :, :])
            nc.vector.memset(va3[:, :, D:D + 1], 1.0)

        # ---- attention group: head pair hp, l-chunk lc -------------------
        def attn(lc_i, hp):
            lcs, lcn = lch[lc_i]
            h1, h2 = 2 * hp, 2 * hp + 1
            pts = []
            for nt in range(NT):
                ps = psS.tile([128, 1024], f32, tag="psS")
                nsl = slice(128 * nt, 128 * (nt + 1))
                nc.tensor.matmul(
                    ps[:, 0:lcn],
                    lhsT=kt_sb[hp][0:64, nsl],
                    rhs=qht_sb[hp][0:64, lcs:lcs + lcn],
                    start=True, stop=True, tile_position=(0, 0))
                nc.tensor.matmul(
                    ps[:, 512:512 + lcn],
                    lhsT=kt_sb[hp][64:128, nsl],
                    rhs=qht_sb[hp][64:128, lcs:lcs + lcn],
                    start=True, stop=True, tile_position=(64, 0))
                pt = ptp.tile([128, 1024], bf16, tag="pt")
                ps2 = ps[:, 0:1024].rearrange("p (b x) -> p b x", b=2)
                pt2 = pt[:, 0:1024].rearrange("p (b x) -> p b x", b=2)
                nc.scalar.activation(pt2[:, :, 0:lcn], ps2[:, :, 0:lcn],
                                     AF.Exp, scale=SCALE)
                pts.append(pt)
                if nt == 1:
                    zacc = work.tile([128, 1024], bf16, tag="zacc")
                    nc.vector.tensor_tensor(zacc[:, :], pts[0][:, 0:1024],
                                            pts[1][:, 0:1024], ALU.add)
                elif nt > 1:
                    nc.vector.tensor_tensor(zacc[:, :], zacc[:, :],
                                            pts[nt][:, 0:1024], ALU.add)
            # O^T accumulated over the 16 key tiles (col-paired heads).
            poz = psOZ.tile([128, 512], f32, tag="psOZ")
            nc.vector.memset(poz[:, :], 0.0)
            for nt in range(NT):
                va3 = vaug_sb[nt][:].rearrange("p (h e) -> p h e", h=H)
                stop = (nt == NT - 1)
                nc.tensor.matmul(
                    poz[0:64, 0:lcn], lhsT=va3[:, h1, 0:D],
                    rhs=pts[nt][:, 0:lcn],
                    start=False, stop=stop, tile_position=(0, 0),
                    skip_group_check=True)
                nc.tensor.matmul(
                    poz[64:128, 0:lcn], lhsT=va3[:, h2, 0:D],
                    rhs=pts[nt][:, 512:512 + lcn],
                    start=False, stop=stop, tile_position=(0, 64),
                    skip_group_check=True)
            # broadcast-sum Z to all partitions: ones^T . zacc
            pbz = psP.tile([128, 512], f32, tag="psP")
            nc.tensor.matmul(pbz[:, 0:lcn], lhsT=ones_sb[:, 0:128],
                             rhs=zacc[:, 0:lcn], start=True, stop=True)
            pbz2 = psP.tile([128, 512], f32, tag="psP")
            nc.tensor.matmul(pbz2[:, 0:lcn], lhsT=ones_sb[:, 0:128],
                             rhs=zacc[:, 512:512 + lcn], start=True, stop=True)
            if dbg and lc_i == 0 and hp == 0:
                dcp2 = work.tile([128, 1024], f32, tag="dcp2")
                nc.vector.tensor_copy(dcp2[:, :], poz[:, 0:1024])
                nc.sync.dma_start(dbg_poz[:, :], dcp2[:, :])
            bz_sb = work.tile([128, 1024], f32, tag="bz")
            nc.vector.reciprocal_approx_fast(bz_sb[:, 0:lcn], pbz[:, 0:lcn])
            nc.vector.reciprocal_approx_fast(bz_sb[:, 512:512 + lcn],
                                             pbz2[:, 0:lcn])
            onrm = work.tile([128, 512], bf16, tag=f"onrm{hp}")
            nc.vector.tensor_tensor(onrm[0:64, 0:lcn], poz[0:64, 0:lcn],
                                    bz_sb[0:64, 0:lcn], ALU.mult)
            nc.vector.tensor_tensor(onrm[64:128, 0:lcn], poz[64:128, 0:lcn],
                                    bz_sb[64:128, 512:512 + lcn], ALU.mult)
            if dbg and lc_i == 0 and hp == 0:
                dcp3 = work.tile([128, 512], f32, tag="dcp3")
                nc.vector.tensor_copy(dcp3[:, :], onrm[:, 0:512])
                nc.sync.dma_start(dbg_onrm[:, :], dcp3[:, :])
            return onrm

        def proj_out(lc_i, onrms):
            lcs, lcn = lch[lc_i]
            for jt in range(CT):
                py = psOZ.tile([128, 512], f32, tag="psOZ")
                ys = work.tile([128, 512], f32, tag="ys")
                for hp in range(HPAIRS):
                    nc.tensor.matmul(
                        py[:, 0:lcn],
                        lhsT=wp_sb[hp][:, 128 * jt:128 * (jt + 1)],
                        rhs=onrms[hp][:, 0:lcn],
                        start=(hp == 0), stop=(hp == HPAIRS - 1))
                nc.vector.tensor_scalar(
                    ys[:, 0:lcn], py[:, 0:lcn],
                    bias_sb[:, jt:jt + 1], None, ALU.add)
                nc.sync.dma_start(
                    outT[128 * jt:128 * (jt + 1), lcs:lcs + lcn],
                    ys[:, 0:lcn])

        # ---- emission order (scheduling priority) ------------------------
        proj_kt(0)
        proj_qht(0)
        proj_kt(1)
        proj_qht(1)
        # Remaining projections at background priority: they run in PE gaps
        # of the ACT-bound exp stream instead of serializing up front.
        with tc.high_priority(offset=-(10 ** 6)):
            for nt in range(NT):
                proj_v(nt)
            for jt in range(2, HPAIRS):
                proj_kt(jt)
                proj_qht(jt)
        if dbg:
            dk = work.tile([128, NKEY], f32, tag="dk", bufs=1)
            nc.vector.tensor_copy(dk[:, :], kt_sb[0][:, :])
            nc.sync.dma_start(dbg_kt[:, :], dk[:, :])
            dq = work.tile([128, L], f32, tag="dq", bufs=1)
            nc.vector.tensor_copy(dq[:, :], qht_sb[0][:, :])
            nc.sync.dma_start(dbg_qht[:, :], dq[:, :])
        pending = None
        for lc_i in range(len(lch)):
            onrms = []
            for hp in range(HPAIRS):
                onrms.append(attn(lc_i, hp))
                if hp == 0 and pending is not None:
                    with tc.high_priority(offset=-(10 ** 5)):
                        proj_out(*pending)
                    pending = None
            pending = (lc_i, onrms)
        proj_out(*pending)

    nc.compile()
    _BUILD_CACHE[key] = nc
    return nc


def kernel(x, q, Wq, Wkv, Wproj, bproj, q_lengths, max_q_len):
    import ml_dtypes
    from concourse.bass_utils import run_bass_kernel_spmd

    bf16 = ml_dtypes.bfloat16
    x = np.asarray(x, np.float32)
    q = np.asarray(q, np.float32)
    Wq = np.asarray(Wq, np.float32)
    Wkv = np.asarray(Wkv, np.float32)
    Wproj = np.asarray(Wproj, np.float32)
    bproj = np.asarray(bproj, np.float32)
    q_lengths = np.asarray(q_lengths, np.int64)
    assert x.shape[0] == NCORES == B

    L = int(((q_lengths.max() + 127) // 128) * 128)
    nc = _build(L)

    offs = np.concatenate([[0], np.cumsum(q_lengths)])
    wqT = np.ascontiguousarray(Wq.T).astype(bf16)
    wkT = np.ascontiguousarray(Wkv[:C].T).astype(bf16)
    wvT = np.ascontiguousarray(Wkv[C:].T).astype(bf16)
    wpT = np.ascontiguousarray(Wproj.T).astype(bf16)
    biasP = np.ascontiguousarray(bproj.reshape(CT, 128).T).astype(np.float32)

    in_maps = []
    for b in range(B):
        Lb = int(q_lengths[b])
        qseg = q[offs[b]:offs[b] + Lb]
        qTp = np.zeros((C, L), bf16)
        qTp[:, :Lb] = qseg.T.astype(bf16)
        in_maps.append({
            "xT": np.ascontiguousarray(x[b].T).astype(bf16),
            "qT": qTp,
            "wqT": wqT, "wkT": wkT, "wvT": wvT, "wpT": wpT,
            "biasP": biasP,
        })

    trace = os.environ.get("KERNEL_TRACE", "") == "1"
    if trace:
        try:
            import sys
            import types
            import antenv
            if "antenv.axon_hooks" not in sys.modules:
                from trn_agent_boot.trn_boot import _ntff_profile_via_ctypes
                hook = _ntff_profile_via_ctypes("/opt/axon/libaxon_pjrt.so")
                mod = types.ModuleType("antenv.axon_hooks")
                mod.get_axon_ntff_profile_hook = lambda: hook
                sys.modules["antenv.axon_hooks"] = mod
                antenv.axon_hooks = mod
        except Exception as e:
            print(f"ntff hook setup failed: {e}")
            trace = False
    res = run_bass_kernel_spmd(nc, in_maps, core_ids=list(range(NCORES)),
                               trace=trace)
    if trace and res.exec_time_ns is not None:
        print(f"HW exec time: {res.exec_time_ns} ns")
        if res.instructions_and_trace:
            print(f"trace: {res.instructions_and_trace[1]}")

    out = np.empty((int(offs[-1]), C), np.float32)
    for b in range(B):
        Lb = int(q_lengths[b])
        out[offs[b]:offs[b] + Lb] = res.results[b]["out"][:, :Lb].T
    return out



# revision 25
# speedup vs baseline: 2.0574x; 1.0002x over previous
"""Trainium2 Bass kernel: varlen batched cross-attention (sparse_attention).

Math (per reference):
  qh = q @ Wq.T           [Tq, H, D]
  k,v = split(x @ Wkv.T)  [B, N, H, D]
  per batch b: queries of segment b attend over batch b's N keys
  out = softmax(qh k^T / sqrt(D)) v  -> [Tq, C] @ Wproj.T + bproj

Sharding: batch-parallel over 8 cores (core b owns batch b), zero
collectives. Host pre-transposes all operands so every device matmul
contracts over the partition axis. All queries padded to a uniform L
(multiple of 128) so one NEFF serves all cores.

Device layout (per core):
  xT [C, N], qT [C, L] bf16  (feature-major)
  K^T computed as head-pair tiles kt[hp] [128, N]  (d on partitions)
  S^T = K^T_h . qhT_h  per 128-key tile -> exp on ScalarE (scale fused)
  O^T + Z via V-augmented (ones col) matmuls, col-paired heads
  normalize with 1/Z broadcast through a tiny PE matmul
  y^T = Wproj^T . O^T + bias -> DMA out [C, L] f32; host transposes back
"""

import os
import numpy as np

B, NKEY, C, H, D = 8, 2048, 512, 8, 64
NCORES = 8
CT = C // 128          # 4 c-tiles
NT = NKEY // 128       # 16 key tiles
HPAIRS = H // 2        # 4 head pairs
SCALE = float(D) ** -0.5

_BUILD_CACHE = {}


def _lchunks(L):
    out = []
    off = 0
    while off < L:
        sz = min(512, L - off)
        out.append((off, sz))
        off += sz
    return out


def _build(L, dbg=False):
    key = (L, dbg)
    if key in _BUILD_CACHE:
        return _BUILD_CACHE[key]
    from contextlib import ExitStack
    import concourse.bass as bass
    import concourse.tile as tile
    import concourse.mybir as mybir
    from concourse import bacc

    f32 = mybir.dt.float32
    bf16 = mybir.dt.bfloat16
    AF = mybir.ActivationFunctionType
    ALU = mybir.AluOpType

    lch = _lchunks(L)

    nc = bacc.Bacc("TRN2", target_bir_lowering=False, debug=False)
    xT = nc.declare_dram_parameter("xT", [C, NKEY], bf16, isOutput=False)
    qT = nc.declare_dram_parameter("qT", [C, L], bf16, isOutput=False)
    wqT = nc.declare_dram_parameter("wqT", [C, C], bf16, isOutput=False)
    wkT = nc.declare_dram_parameter("wkT", [C, C], bf16, isOutput=False)
    wvT = nc.declare_dram_parameter("wvT", [C, C], bf16, isOutput=False)
    wpT = nc.declare_dram_parameter("wpT", [C, C], bf16, isOutput=False)
    biasP = nc.declare_dram_parameter("biasP", [128, CT], f32, isOutput=False)
    outT = nc.declare_dram_parameter("out", [C, L], f32, isOutput=True)
    if dbg:
        dbg_kt = nc.declare_dram_parameter("dbg_kt", [128, NKEY], f32, isOutput=True)
        dbg_qht = nc.declare_dram_parameter("dbg_qht", [128, L], f32, isOutput=True)
        dbg_pt = nc.declare_dram_parameter("dbg_pt", [128, 1024], f32, isOutput=True)
        dbg_poz = nc.declare_dram_parameter("dbg_poz", [128, 1024], f32, isOutput=True)
        dbg_onrm = nc.declare_dram_parameter("dbg_onrm", [128, 512], f32, isOutput=True)

    with ExitStack() as ctx:
        tc = ctx.enter_context(tile.TileContext(nc))
        pers = ctx.enter_context(tc.tile_pool(name="pers", bufs=1))
        psS = ctx.enter_context(tc.tile_pool(name="psS", bufs=2, space="PSUM"))
        psOZ = ctx.enter_context(tc.tile_pool(name="psOZ", bufs=2, space="PSUM"))
        psP = ctx.enter_context(tc.tile_pool(name="psP", bufs=2, space="PSUM"))
        ptp = ctx.enter_context(tc.tile_pool(name="ptp", bufs=32))
        work = ctx.enter_context(tc.tile_pool(name="work", bufs=3))

        # ---- persistent inputs -------------------------------------------
        xt_sb = [pers.tile([128, NKEY], bf16, tag=f"xt{i}", name=f"xt{i}") for i in range(CT)]
        qt_sb = [pers.tile([128, L], bf16, tag=f"qt{i}", name=f"qt{i}") for i in range(CT)]
        wq_sb = [pers.tile([128, C], bf16, tag=f"wq{i}", name=f"wq{i}") for i in range(CT)]
        wk_sb = [pers.tile([128, C], bf16, tag=f"wk{i}", name=f"wk{i}") for i in range(CT)]
        wv_sb = [pers.tile([128, C], bf16, tag=f"wv{i}", name=f"wv{i}") for i in range(CT)]
        wp_sb = [pers.tile([128, C], bf16, tag=f"wp{i}", name=f"wp{i}") for i in range(CT)]
        for i in range(CT):
            sl = slice(128 * i, 128 * (i + 1))
            nc.sync.dma_start(xt_sb[i][:], xT[sl, :])
            nc.sync.dma_start(wk_sb[i][:], wkT[sl, :])
            nc.sync.dma_start(wv_sb[i][:], wvT[sl, :])
        for i in range(CT):
            sl = slice(128 * i, 128 * (i + 1))
            nc.sync.dma_start(wq_sb[i][:], wqT[sl, :])
            nc.sync.dma_start(qt_sb[i][:], qT[sl, :])
            nc.sync.dma_start(wp_sb[i][:], wpT[sl, :])
        bias_sb = pers.tile([128, CT], f32, tag="bias")
        nc.sync.dma_start(bias_sb[:], biasP[:])

        # all-ones matrix: matmul(lhsT=ones, rhs=zacc) broadcasts the
        # partition-colsum of zacc to every output partition in one shot
        ones_sb = pers.tile([128, 128], bf16, tag="ones")
        nc.vector.memset(ones_sb[:], 1.0)

        # ---- projections --------------------------------------------------
        kt_sb = [pers.tile([128, NKEY], bf16, tag=f"kt{i}", name=f"kt{i}") for i in range(HPAIRS)]
        qht_sb = [pers.tile([128, L], bf16, tag=f"qht{i}", name=f"qht{i}") for i in range(HPAIRS)]
        vaug_sb = [pers.tile([128, H * (D + 1)], bf16, tag=f"va{i}", name=f"va{i}") for i in range(NT)]

        def proj_kt(jt):
            for nch in range(NKEY // 512):
                ps = psP.tile([128, 512], f32, tag="psP")
                for ct in range(CT):
                    nc.tensor.matmul(
                        ps[:, 0:512],
                        lhsT=wk_sb[ct][:, 128 * jt:128 * (jt + 1)],
                        rhs=xt_sb[ct][:, 512 * nch:512 * (nch + 1)],
                        start=(ct == 0), stop=(ct == CT - 1))
                nc.vector.tensor_copy(
                    kt_sb[jt][:, 512 * nch:512 * (nch + 1)], ps[:, 0:512])

        def proj_qht(jt):
            for (lcs, lcn) in lch:
                ps = psP.tile([128, 512], f32, tag="psP")
                for ct in range(CT):
                    nc.tensor.matmul(
                        ps[:, 0:lcn],
                        lhsT=wq_sb[ct][:, 128 * jt:128 * (jt + 1)],
                        rhs=qt_sb[ct][:, lcs:lcs + lcn],
                        start=(ct == 0), stop=(ct == CT - 1))
                nc.vector.tensor_copy(qht_sb[jt][:, lcs:lcs + lcn], ps[:, 0:lcn])

        def proj_v(nt):
            ps = psP.tile([128, 512], f32, tag="psP")
            for ct in range(CT):
                nc.tensor.matmul(
                    ps[:, 0:512],
                    lhsT=xt_sb[ct][:, 128 * nt:128 * (nt + 1)],
                    rhs=wv_sb[ct][:, 0:C],
                    start=(ct == 0), stop=(ct == CT - 1))
            va3 = vaug_sb[nt][:].rearrange("p (h e) -> p h e", h=H)
            ps3 = ps[:, 0:512].rearrange("p (h d) -> p h d", h=H)
            nc.vector.tensor_copy(va3[:, :, 0:D], ps3[:, :, :])
            nc.vector.memset(va3[:, :, D:D + 1], 1.0)

        # ---- attention group: head pair hp, l-chunk lc -------------------
        def attn(lc_i, hp):
            lcs, lcn = lch[lc_i]
            h1, h2 = 2 * hp, 2 * hp + 1
            pts = []
            for nt in range(NT):
                ps = psS.tile([128, 1024], f32, tag="psS")
                nsl = slice(128 * nt, 128 * (nt + 1))
                nc.tensor.matmul(
                    ps[:, 0:lcn],
                    lhsT=kt_sb[hp][0:64, nsl],
                    rhs=qht_sb[hp][0:64, lcs:lcs + lcn],
                    start=True, stop=True, tile_position=(0, 0))
                nc.tensor.matmul(
                    ps[:, 512:512 + lcn],
                    lhsT=kt_sb[hp][64:128, nsl],
                    rhs=qht_sb[hp][64:128, lcs:lcs + lcn],
                    start=True, stop=True, tile_position=(64, 0))
                pt = ptp.tile([128, 1024], bf16, tag="pt")
                ps2 = ps[:, 0:1024].rearrange("p (b x) -> p b x", b=2)
                pt2 = pt[:, 0:1024].rearrange("p (b x) -> p b x", b=2)
                nc.scalar.activation(pt2[:, :, 0:lcn], ps2[:, :, 0:lcn],
                                     AF.Exp, scale=SCALE)
                pts.append(pt)
                if nt == 1:
                    zacc = work.tile([128, 1024], bf16, tag="zacc")
                    nc.vector.tensor_tensor(zacc[:, :], pts[0][:, 0:1024],
                                            pts[1][:, 0:1024], ALU.add)
                elif nt > 1:
                    nc.vector.tensor_tensor(zacc[:, :], zacc[:, :],
                                            pts[nt][:, 0:1024], ALU.add)
            # O^T accumulated over the 16 key tiles (col-paired heads).
            poz = psOZ.tile([128, 512], f32, tag="psOZ")
            nc.vector.memset(poz[:, :], 0.0)
            for nt in range(NT):
                va3 = vaug_sb[nt][:].rearrange("p (h e) -> p h e", h=H)
                stop = (nt == NT - 1)
                nc.tensor.matmul(
                    poz[0:64, 0:lcn], lhsT=va3[:, h1, 0:D],
                    rhs=pts[nt][:, 0:lcn],
                    start=False, stop=stop, tile_position=(0, 0),
                    skip_group_check=True)
                nc.tensor.matmul(
                    poz[64:128, 0:lcn], lhsT=va3[:, h2, 0:D],
                    rhs=pts[nt][:, 512:512 + lcn],
                    start=False, stop=stop, tile_position=(0, 64),
                    skip_group_check=True)
            # broadcast-sum Z to all partitions: ones^T . zacc
            pbz = psP.tile([128, 512], f32, tag="psP")
            nc.tensor.matmul(pbz[:, 0:lcn], lhsT=ones_sb[:, 0:128],
                             rhs=zacc[:, 0:lcn], start=True, stop=True)
            pbz2 = psP.tile([128, 512], f32, tag="psP")
            nc.tensor.matmul(pbz2[:, 0:lcn], lhsT=ones_sb[:, 0:128],
                             rhs=zacc[:, 512:512 + lcn], start=True, stop=True)
            if dbg and lc_i == 0 and hp == 0:
                dcp2 = work.tile([128, 1024], f32, tag="dcp2")
                nc.vector.tensor_copy(dcp2[:, :], poz[:, 0:1024])
                nc.sync.dma_start(dbg_poz[:, :], dcp2[:, :])
            bz_sb = work.tile([128, 1024], f32, tag="bz")
            nc.vector.reciprocal_approx_fast(bz_sb[:, 0:lcn], pbz[:, 0:lcn])
            nc.vector.reciprocal_approx_fast(bz_sb[:, 512:512 + lcn],
                                             pbz2[:, 0:lcn])
            onrm = work.tile([128, 512], bf16, tag=f"onrm{hp}")
            nc.vector.tensor_tensor(onrm[0:64, 0:lcn], poz[0:64, 0:lcn],
                                    bz_sb[0:64, 0:lcn], ALU.mult)
            nc.vector.tensor_tensor(onrm[64:128, 0:lcn], poz[64:128, 0:lcn],
                                    bz_sb[64:128, 512:512 + lcn], ALU.mult)
            if dbg and lc_i == 0 and hp == 0:
                dcp3 = work.tile([128, 512], f32, tag="dcp3")
                nc.vector.tensor_copy(dcp3[:, :], onrm[:, 0:512])
                nc.sync.dma_start(dbg_onrm[:, :], dcp3[:, :])
            return onrm

        def proj_out(lc_i, onrms):
            lcs, lcn = lch[lc_i]
            for jt in range(CT):
                py = psOZ.tile([128, 512], f32, tag="psOZ")
                ys = work.tile([128, 512], f32, tag="ys")
                for hp in range(HPAIRS):
                    nc.tensor.matmul(
                        py[:, 0:lcn],
                        lhsT=wp_sb[hp][:, 128 * jt:128 * (jt + 1)],
                        rhs=onrms[hp][:, 0:lcn],
                        start=(hp == 0), stop=(hp == HPAIRS - 1))
                nc.vector.tensor_scalar(
                    ys[:, 0:lcn], py[:, 0:lcn],
                    bias_sb[:, jt:jt + 1], None, ALU.add)
                nc.sync.dma_start(
                    outT[128 * jt:128 * (jt + 1), lcs:lcs + lcn],
                    ys[:, 0:lcn])

        # ---- emission order (scheduling priority) ------------------------
        proj_kt(0)
        proj_qht(0)
        proj_kt(1)
        proj_qht(1)
        # Remaining projections at background priority: they run in PE gaps
        # of the ACT-bound exp stream instead of serializing up front.
        with tc.high_priority(offset=-(10 ** 6)):
            for nt in range(NT):
                proj_v(nt)
            for jt in range(2, HPAIRS):
                proj_kt(jt)
                proj_qht(jt)
        if dbg:
            dk = work.tile([128, NKEY], f32, tag="dk", bufs=1)
            nc.vector.tensor_copy(dk[:, :], kt_sb[0][:, :])
            nc.sync.dma_start(dbg_kt[:, :], dk[:, :])
            dq = work.tile([128, L], f32, tag="dq", bufs=1)
            nc.vector.tensor_copy(dq[:, :], qht_sb[0][:, :])
            nc.sync.dma_start(dbg_qht[:, :], dq[:, :])
        pending = None
        for lc_i in range(len(lch)):
            onrms = []
            for hp in range(HPAIRS):
                onrms.append(attn(lc_i, hp))
                if hp == 0 and pending is not None:
                    with tc.high_priority(offset=-(10 ** 5)):
                        proj_out(*pending)
                    pending = None
            pending = (lc_i, onrms)
        proj_out(*pending)

    nc.compile()
    _BUILD_CACHE[key] = nc
    return nc


def kernel(x, q, Wq, Wkv, Wproj, bproj, q_lengths, max_q_len):
    import ml_dtypes
    from concourse.bass_utils import run_bass_kernel_spmd

    bf16 = ml_dtypes.bfloat16
    x = np.asarray(x, np.float32)
    q = np.asarray(q, np.float32)
    Wq = np.asarray(Wq, np.float32)
    Wkv = np.asarray(Wkv, np.float32)
    Wproj = np.asarray(Wproj, np.float32)
    bproj = np.asarray(bproj, np.float32)
    q_lengths = np.asarray(q_lengths, np.int64)
    assert x.shape[0] == NCORES == B

    L = int(((q_lengths.max() + 127) // 128) * 128)
    nc = _build(L)

    offs = np.concatenate([[0], np.cumsum(q_lengths)])
    wqT = np.ascontiguousarray(Wq.T).astype(bf16)
    wkT = np.ascontiguousarray(Wkv[:C].T).astype(bf16)
    wvT = np.ascontiguousarray(Wkv[C:].T).astype(bf16)
    wpT = np.ascontiguousarray(Wproj.T).astype(bf16)
    biasP = np.ascontiguousarray(bproj.reshape(CT, 128).T).astype(np.float32)

    in_maps = []
    for b in range(B):
        Lb = int(q_lengths[b])
        qseg = q[offs[b]:offs[b] + Lb]
        qTp = np.zeros((C, L), bf16)
        qTp[:, :Lb] = qseg.T.astype(bf16)
        in_maps.append({
            "xT": np.ascontiguousarray(x[b].T).astype(bf16),
            "qT": qTp,
            "wqT": wqT, "wkT": wkT, "wvT": wvT, "wpT": wpT,
            "biasP": biasP,
        })

    trace = os.environ.get("KERNEL_TRACE", "") == "1"
    if trace:
        try:
            import sys
            import types
            import antenv
            if "antenv.axon_hooks" not in sys.modules:
                from trn_agent_boot.trn_boot import _ntff_profile_via_ctypes
                hook = _ntff_profile_via_ctypes("/opt/axon/libaxon_pjrt.so")
                mod = types.ModuleType("antenv.axon_hooks")
                mod.get_axon_ntff_profile_hook = lambda: hook
                sys.modules["antenv.axon_hooks"] = mod
                antenv.axon_hooks = mod
        except Exception as e:
            print(f"ntff hook setup failed: {e}")
            trace = False
    res = run_bass_kernel_spmd(nc, in_maps, core_ids=list(range(NCORES)),
                               trace=trace)
    if trace and res.exec_time_ns is not None:
        print(f"HW exec time: {res.exec_time_ns} ns")
        if res.instructions_and_trace:
            print(f"trace: {res.instructions_and_trace[1]}")

    out = np.empty((int(offs[-1]), C), np.float32)
    for b in range(B):
        Lb = int(q_lengths[b])
        out[offs[b]:offs[b] + Lb] = res.results[b]["out"][:, :Lb].T
    return out



# revision 28
# speedup vs baseline: 2.0682x; 1.0052x over previous
"""Trainium2 Bass kernel: varlen batched cross-attention (sparse_attention).

Math (per reference):
  qh = q @ Wq.T           [Tq, H, D]
  k,v = split(x @ Wkv.T)  [B, N, H, D]
  per batch b: queries of segment b attend over batch b's N keys
  out = softmax(qh k^T / sqrt(D)) v  -> [Tq, C] @ Wproj.T + bproj

Sharding: batch-parallel over 8 cores (core b owns batch b), zero
collectives. Host pre-transposes all operands so every device matmul
contracts over the partition axis. All queries padded to a uniform L
(multiple of 128) so one NEFF serves all cores.

Device layout (per core):
  xT [C, N], qT [C, L] bf16  (feature-major)
  K^T computed as head-pair tiles kt[hp] [128, N]  (d on partitions)
  S^T = K^T_h . qhT_h  per 128-key tile -> exp on ScalarE (scale fused)
  O^T + Z via V-augmented (ones col) matmuls, col-paired heads
  normalize with 1/Z broadcast through a tiny PE matmul
  y^T = Wproj^T . O^T + bias -> DMA out [C, L] f32; host transposes back
"""

import os
import numpy as np

B, NKEY, C, H, D = 8, 2048, 512, 8, 64
NCORES = 8
CT = C // 128          # 4 c-tiles
NT = NKEY // 128       # 16 key tiles
HPAIRS = H // 2        # 4 head pairs
SCALE = float(D) ** -0.5

_BUILD_CACHE = {}


def _lchunks(L):
    out = []
    off = 0
    while off < L:
        sz = min(512, L - off)
        out.append((off, sz))
        off += sz
    return out


def _build(L, dbg=False):
    key = (L, dbg)
    if key in _BUILD_CACHE:
        return _BUILD_CACHE[key]
    from contextlib import ExitStack
    import concourse.bass as bass
    import concourse.tile as tile
    import concourse.mybir as mybir
    from concourse import bacc

    f32 = mybir.dt.float32
    bf16 = mybir.dt.bfloat16
    AF = mybir.ActivationFunctionType
    ALU = mybir.AluOpType

    lch = _lchunks(L)

    nc = bacc.Bacc("TRN2", target_bir_lowering=False, debug=False)
    xT = nc.declare_dram_parameter("xT", [C, NKEY], bf16, isOutput=False)
    qT = nc.declare_dram_parameter("qT", [C, L], bf16, isOutput=False)
    wqT = nc.declare_dram_parameter("wqT", [C, C], bf16, isOutput=False)
    wkT = nc.declare_dram_parameter("wkT", [C, C], bf16, isOutput=False)
    wvT = nc.declare_dram_parameter("wvT", [C, C], bf16, isOutput=False)
    wpT = nc.declare_dram_parameter("wpT", [C, C], bf16, isOutput=False)
    biasP = nc.declare_dram_parameter("biasP", [128, CT], f32, isOutput=False)
    outT = nc.declare_dram_parameter("out", [C, L], f32, isOutput=True)
    if dbg:
        dbg_kt = nc.declare_dram_parameter("dbg_kt", [128, NKEY], f32, isOutput=True)
        dbg_qht = nc.declare_dram_parameter("dbg_qht", [128, L], f32, isOutput=True)
        dbg_pt = nc.declare_dram_parameter("dbg_pt", [128, 1024], f32, isOutput=True)
        dbg_poz = nc.declare_dram_parameter("dbg_poz", [128, 1024], f32, isOutput=True)
        dbg_onrm = nc.declare_dram_parameter("dbg_onrm", [128, 512], f32, isOutput=True)

    with ExitStack() as ctx:
        tc = ctx.enter_context(tile.TileContext(nc))
        pers = ctx.enter_context(tc.tile_pool(name="pers", bufs=1))
        psS = ctx.enter_context(tc.tile_pool(name="psS", bufs=2, space="PSUM"))
        psOZ = ctx.enter_context(tc.tile_pool(name="psOZ", bufs=2, space="PSUM"))
        psP = ctx.enter_context(tc.tile_pool(name="psP", bufs=2, space="PSUM"))
        ptp = ctx.enter_context(tc.tile_pool(name="ptp", bufs=30))
        work = ctx.enter_context(tc.tile_pool(name="work", bufs=3))

        # ---- persistent inputs -------------------------------------------
        xt_sb = [pers.tile([128, NKEY], bf16, tag=f"xt{i}", name=f"xt{i}") for i in range(CT)]
        qt_sb = [pers.tile([128, L], bf16, tag=f"qt{i}", name=f"qt{i}") for i in range(CT)]
        wq_sb = [pers.tile([128, C], bf16, tag=f"wq{i}", name=f"wq{i}") for i in range(CT)]
        wk_sb = [pers.tile([128, C], bf16, tag=f"wk{i}", name=f"wk{i}") for i in range(CT)]
        wv_sb = [pers.tile([128, C], bf16, tag=f"wv{i}", name=f"wv{i}") for i in range(CT)]
        wp_sb = [pers.tile([128, C], bf16, tag=f"wp{i}", name=f"wp{i}") for i in range(CT)]
        for i in range(CT):
            sl = slice(128 * i, 128 * (i + 1))
            nc.sync.dma_start(xt_sb[i][:], xT[sl, :])
            nc.sync.dma_start(wk_sb[i][:], wkT[sl, :])
            nc.sync.dma_start(wv_sb[i][:], wvT[sl, :])
        for i in range(CT):
            sl = slice(128 * i, 128 * (i + 1))
            nc.sync.dma_start(wq_sb[i][:], wqT[sl, :])
            nc.sync.dma_start(qt_sb[i][:], qT[sl, :])
            nc.sync.dma_start(wp_sb[i][:], wpT[sl, :])
        bias_sb = pers.tile([128, CT], f32, tag="bias")
        nc.sync.dma_start(bias_sb[:], biasP[:])

        # all-ones matrix: matmul(lhsT=ones, rhs=zacc) broadcasts the
        # partition-colsum of zacc to every output partition in one shot
        ones_sb = pers.tile([128, 128], bf16, tag="ones")
        nc.vector.memset(ones_sb[:], 1.0)

        # ---- projections --------------------------------------------------
        kt_sb = [pers.tile([128, NKEY], bf16, tag=f"kt{i}", name=f"kt{i}") for i in range(HPAIRS)]
        qht_sb = [pers.tile([128, L], bf16, tag=f"qht{i}", name=f"qht{i}") for i in range(HPAIRS)]
        vaug_sb = [pers.tile([128, H * (D + 1)], bf16, tag=f"va{i}", name=f"va{i}") for i in range(NT)]

        def proj_kt(jt):
            for nch in range(NKEY // 512):
                ps = psP.tile([128, 512], f32, tag="psP")
                for ct in range(CT):
                    nc.tensor.matmul(
                        ps[:, 0:512],
                        lhsT=wk_sb[ct][:, 128 * jt:128 * (jt + 1)],
                        rhs=xt_sb[ct][:, 512 * nch:512 * (nch + 1)],
                        start=(ct == 0), stop=(ct == CT - 1))
                nc.vector.tensor_copy(
                    kt_sb[jt][:, 512 * nch:512 * (nch + 1)], ps[:, 0:512])

        def proj_qht(jt):
            for (lcs, lcn) in lch:
                ps = psP.tile([128, 512], f32, tag="psP")
                for ct in range(CT):
                    nc.tensor.matmul(
                        ps[:, 0:lcn],
                        lhsT=wq_sb[ct][:, 128 * jt:128 * (jt + 1)],
                        rhs=qt_sb[ct][:, lcs:lcs + lcn],
                        start=(ct == 0), stop=(ct == CT - 1))
                nc.vector.tensor_copy(qht_sb[jt][:, lcs:lcs + lcn], ps[:, 0:lcn])

        def proj_v(nt):
            ps = psP.tile([128, 512], f32, tag="psP")
            for ct in range(CT):
                nc.tensor.matmul(
                    ps[:, 0:512],
                    lhsT=xt_sb[ct][:, 128 * nt:128 * (nt + 1)],
                    rhs=wv_sb[ct][:, 0:C],
                    start=(ct == 0), stop=(ct == CT - 1))
            va3 = vaug_sb[nt][:].rearrange("p (h e) -> p h e", h=H)
            ps3 = ps[:, 0:512].rearrange("p (h d) -> p h d", h=H)
            nc.vector.tensor_copy(va3[:, :, 0:D], ps3[:, :, :])
            nc.vector.memset(va3[:, :, D:D + 1], 1.0)

        # ---- attention group: head pair hp, l-chunk lc -------------------
        def attn(lc_i, hp):
            lcs, lcn = lch[lc_i]
            h1, h2 = 2 * hp, 2 * hp + 1
            pts = []
            for nt in range(NT):
                ps = psS.tile([128, 1024], f32, tag="psS")
                nsl = slice(128 * nt, 128 * (nt + 1))
                nc.tensor.matmul(
                    ps[:, 0:lcn],
                    lhsT=kt_sb[hp][0:64, nsl],
                    rhs=qht_sb[hp][0:64, lcs:lcs + lcn],
                    start=True, stop=True, tile_position=(0, 0))
                nc.tensor.matmul(
                    ps[:, 512:512 + lcn],
                    lhsT=kt_sb[hp][64:128, nsl],
                    rhs=qht_sb[hp][64:128, lcs:lcs + lcn],
                    start=True, stop=True, tile_position=(64, 0))
                pt = ptp.tile([128, 1024], bf16, tag="pt")
                ps2 = ps[:, 0:1024].rearrange("p (b x) -> p b x", b=2)
                pt2 = pt[:, 0:1024].rearrange("p (b x) -> p b x", b=2)
                nc.scalar.activation(pt2[:, :, 0:lcn], ps2[:, :, 0:lcn],
                                     AF.Exp, scale=SCALE)
                pts.append(pt)
                if nt == 1:
                    zacc = work.tile([128, 1024], bf16, tag="zacc")
                    nc.vector.tensor_tensor(zacc[:, :], pts[0][:, 0:1024],
                                            pts[1][:, 0:1024], ALU.add)
                elif nt > 1:
                    nc.vector.tensor_tensor(zacc[:, :], zacc[:, :],
                                            pts[nt][:, 0:1024], ALU.add)
            # O^T accumulated over the 16 key tiles (col-paired heads).
            poz = psOZ.tile([128, 512], f32, tag="psOZ")
            nc.vector.memset(poz[:, :], 0.0)
            for nt in range(NT):
                va3 = vaug_sb[nt][:].rearrange("p (h e) -> p h e", h=H)
                stop = (nt == NT - 1)
                nc.tensor.matmul(
                    poz[0:64, 0:lcn], lhsT=va3[:, h1, 0:D],
                    rhs=pts[nt][:, 0:lcn],
                    start=False, stop=stop, tile_position=(0, 0),
                    skip_group_check=True)
                nc.tensor.matmul(
                    poz[64:128, 0:lcn], lhsT=va3[:, h2, 0:D],
                    rhs=pts[nt][:, 512:512 + lcn],
                    start=False, stop=stop, tile_position=(0, 64),
                    skip_group_check=True)
            # broadcast-sum Z to all partitions: ones^T . zacc
            pbz = psP.tile([128, 512], f32, tag="psP")
            nc.tensor.matmul(pbz[:, 0:lcn], lhsT=ones_sb[:, 0:128],
                             rhs=zacc[:, 0:lcn], start=True, stop=True)
            pbz2 = psP.tile([128, 512], f32, tag="psP")
            nc.tensor.matmul(pbz2[:, 0:lcn], lhsT=ones_sb[:, 0:128],
                             rhs=zacc[:, 512:512 + lcn], start=True, stop=True)
            if dbg and lc_i == 0 and hp == 0:
                dcp2 = work.tile([128, 1024], f32, tag="dcp2")
                nc.vector.tensor_copy(dcp2[:, :], poz[:, 0:1024])
                nc.sync.dma_start(dbg_poz[:, :], dcp2[:, :])
            bz_sb = work.tile([128, 1024], f32, tag="bz")
            nc.vector.reciprocal_approx_fast(bz_sb[:, 0:lcn], pbz[:, 0:lcn])
            nc.vector.reciprocal_approx_fast(bz_sb[:, 512:512 + lcn],
                                             pbz2[:, 0:lcn])
            onrm = work.tile([128, 512], bf16, tag=f"onrm{hp}")
            nc.vector.tensor_tensor(onrm[0:64, 0:lcn], poz[0:64, 0:lcn],
                                    bz_sb[0:64, 0:lcn], ALU.mult)
            nc.vector.tensor_tensor(onrm[64:128, 0:lcn], poz[64:128, 0:lcn],
                                    bz_sb[64:128, 512:512 + lcn], ALU.mult)
            if dbg and lc_i == 0 and hp == 0:
                dcp3 = work.tile([128, 512], f32, tag="dcp3")
                nc.vector.tensor_copy(dcp3[:, :], onrm[:, 0:512])
                nc.sync.dma_start(dbg_onrm[:, :], dcp3[:, :])
            return onrm

        def proj_out(lc_i, onrms):
            lcs, lcn = lch[lc_i]
            for jt in range(CT):
                py = psOZ.tile([128, 512], f32, tag="psOZ")
                ys = work.tile([128, 512], f32, tag="ys")
                for hp in range(HPAIRS):
                    nc.tensor.matmul(
                        py[:, 0:lcn],
                        lhsT=wp_sb[hp][:, 128 * jt:128 * (jt + 1)],
                        rhs=onrms[hp][:, 0:lcn],
                        start=(hp == 0), stop=(hp == HPAIRS - 1))
                nc.vector.tensor_scalar(
                    ys[:, 0:lcn], py[:, 0:lcn],
                    bias_sb[:, jt:jt + 1], None, ALU.add)
                nc.sync.dma_start(
                    outT[128 * jt:128 * (jt + 1), lcs:lcs + lcn],
                    ys[:, 0:lcn])

        # ---- emission order (scheduling priority) ------------------------
        proj_kt(0)
        proj_qht(0)
        proj_kt(1)
        proj_qht(1)
        # Remaining projections at background priority: they run in PE gaps
        # of the ACT-bound exp stream instead of serializing up front.
        with tc.high_priority(offset=-(10 ** 6)):
            for nt in range(NT):
                proj_v(nt)
            for jt in range(2, HPAIRS):
                proj_kt(jt)
                proj_qht(jt)
        if dbg:
            dk = work.tile([128, NKEY], f32, tag="dk", bufs=1)
            nc.vector.tensor_copy(dk[:, :], kt_sb[0][:, :])
            nc.sync.dma_start(dbg_kt[:, :], dk[:, :])
            dq = work.tile([128, L], f32, tag="dq", bufs=1)
            nc.vector.tensor_copy(dq[:, :], qht_sb[0][:, :])
            nc.sync.dma_start(dbg_qht[:, :], dq[:, :])
        pending = None
        for lc_i in range(len(lch)):
            onrms = []
            for hp in range(HPAIRS):
                onrms.append(attn(lc_i, hp))
                if hp == 0 and pending is not None:
                    with tc.high_priority(offset=-(10 ** 5)):
                        proj_out(*pending)
                    pending = None
            pending = (lc_i, onrms)
        proj_out(*pending)

    nc.compile()
    _BUILD_CACHE[key] = nc
    return nc


def kernel(x, q, Wq, Wkv, Wproj, bproj, q_lengths, max_q_len):
    import ml_dtypes
    from concourse.bass_utils import run_bass_kernel_spmd

    bf16 = ml_dtypes.bfloat16
    x = np.asarray(x, np.float32)
    q = np.asarray(q, np.float32)
    Wq = np.asarray(Wq, np.float32)
    Wkv = np.asarray(Wkv, np.float32)
    Wproj = np.asarray(Wproj, np.float32)
    bproj = np.asarray(bproj, np.float32)
    q_lengths = np.asarray(q_lengths, np.int64)
    assert x.shape[0] == NCORES == B

    L = int(((q_lengths.max() + 127) // 128) * 128)
    nc = _build(L)

    offs = np.concatenate([[0], np.cumsum(q_lengths)])
    wqT = np.ascontiguousarray(Wq.T).astype(bf16)
    wkT = np.ascontiguousarray(Wkv[:C].T).astype(bf16)
    wvT = np.ascontiguousarray(Wkv[C:].T).astype(bf16)
    wpT = np.ascontiguousarray(Wproj.T).astype(bf16)
    biasP = np.ascontiguousarray(bproj.reshape(CT, 128).T).astype(np.float32)

    in_maps = []
    for b in range(B):
        Lb = int(q_lengths[b])
        qseg = q[offs[b]:offs[b] + Lb]
        qTp = np.zeros((C, L), bf16)
        qTp[:, :Lb] = qseg.T.astype(bf16)
        in_maps.append({
            "xT": np.ascontiguousarray(x[b].T).astype(bf16),
            "qT": qTp,
            "wqT": wqT, "wkT": wkT, "wvT": wvT, "wpT": wpT,
            "biasP": biasP,
        })

    trace = os.environ.get("KERNEL_TRACE", "") == "1"
    if trace:
        try:
            import sys
            import types
            import antenv
            if "antenv.axon_hooks" not in sys.modules:
                from trn_agent_boot.trn_boot import _ntff_profile_via_ctypes
                hook = _ntff_profile_via_ctypes("/opt/axon/libaxon_pjrt.so")
                mod = types.ModuleType("antenv.axon_hooks")
                mod.get_axon_ntff_profile_hook = lambda: hook
                sys.modules["antenv.axon_hooks"] = mod
                antenv.axon_hooks = mod
        except Exception as e:
            print(f"ntff hook setup failed: {e}")
            trace = False
    res = run_bass_kernel_spmd(nc, in_maps, core_ids=list(range(NCORES)),
                               trace=trace)
    if trace and res.exec_time_ns is not None:
        print(f"HW exec time: {res.exec_time_ns} ns")
        if res.instructions_and_trace:
            print(f"trace: {res.instructions_and_trace[1]}")

    out = np.empty((int(offs[-1]), C), np.float32)
    for b in range(B):
        Lb = int(q_lengths[b])
        out[offs[b]:offs[b] + Lb] = res.results[b]["out"][:, :Lb].T
    return out

